# revision 26
# baseline (speedup 1.0000x reference)
"""Trainium2 Bass kernel for the sliding-window-attention transformer
(nn_Model_22728966930624).

Sharding: sequence-parallel over 8 NeuronCores. Core c owns tokens
[c*512, (c+1)*512); each layer's K/V are computed over an extended region
with a 256-token halo on each side. Halos are refreshed between layers with
an 8-rank AllGather (bf16) plus partition-id-indexed dynamic DMAs.

v2 changes vs baseline:
- All "broadcast"/stats matmuls (LN stats, mean/rstd broadcast, softmax
  denominator broadcast) run as float32r (1 cycle/row at N>=512) instead of
  fp32 (4 cycles/row).
- LayerNorm gamma/beta are folded into the broadcast matmuls:
  A = g (x) rstd, B = g (x) (-mean*rstd) + b (x) 1, apply = s*A + B
  (2 vector ops per feature tile).
- Attention: per head, 8 full-width [64,128,512] score matmuls (one per
  ext k-block, center blocks deduplicated), one [65,512] PSUM accumulator,
  paired-head denominator broadcast via a K=2 selector fp32r matmul.
- V bias folded into the O-projection bias on the host (bo_eff = bv@Wo+bo).
- Layer reordered for AllGather overlap: Q -> K-center -> V-center ->
  (halo) K-edges -> V-edges -> attention with center k-blocks first.
- Fused epilogues via scalar_tensor_tensor where possible.
"""
import os
import sys
import types

import numpy as np
import ml_dtypes

import concourse.bass as bass
import concourse.mybir as mybir
import concourse.tile as tile
from concourse.alu_op_type import AluOpType
from concourse.bass_utils import run_bass_kernel_spmd

F32 = mybir.dt.float32
F32R = mybir.dt.float32r
BF16 = mybir.dt.bfloat16
AF = mybir.ActivationFunctionType
NPBF16 = ml_dtypes.bfloat16

# model dims
S, D, H, DH, L, FF = 4096, 768, 12, 64, 4, 3072
C, W = 256, 256
P = 8                   # cores
T_OWN = S // P          # 512
T_EXT = T_OWN + 2 * C   # 1024
NJ = D // 128           # 6 feature row-tiles
NJF = FF // 128         # 24
HS = DH + 1             # 65: V head slot width (extra ones column)
KB = 8                  # ext k-blocks of 128 tokens

# bias/gamma column registry (shared host/device)
PER_LAYER_COLS = 72
NB = 12 + L * PER_LAYER_COLS


def col_emb_g(j): return j
def col_emb_b(j): return 6 + j
def lbase(l): return 12 + l * PER_LAYER_COLS
def col_bq(l, j): return lbase(l) + j
def col_bk(l, j): return lbase(l) + 6 + j
def col_bo(l, j): return lbase(l) + 12 + j
def col_bff2(l, j): return lbase(l) + 18 + j
def col_bff1(l, j): return lbase(l) + 24 + j       # j in 0..23


# gb_rows column registry: [2, NLN*768]; row0=gamma, row1=beta
def gb_emb(): return 0
def gb_ln1(l): return (1 + 2 * l) * D
def gb_ln2(l): return (2 + 2 * l) * D


NLN = 1 + 2 * L

_MAX_WAITS = 1


def _split_excess_waits(nc, max_waits=_MAX_WAITS):
    """This walrus build rejects >1 semaphore wait per instruction; move
    extras onto same-engine NoOps inserted just before."""
    n = 0
    for f in nc.m.functions:
        for bb in f.blocks:
            new_insts = []
            for inst in bb.instructions:
                si = inst.sync_info
                if si is not None and si.on_wait and len(si.on_wait) > max_waits:
                    excess = list(si.on_wait[:-max_waits])
                    keep = list(si.on_wait[-max_waits:])
                    for k, w in enumerate(excess):
                        nop = mybir.InstNoOp(name=f"{inst.name}-wsplit{k}")
                        nop.engine = inst.engine
                        nop.sync_info = mybir.SyncInfo(on_wait=[w], on_update=[])
                        new_insts.append(nop)
                        n += 1
                    inst.sync_info = mybir.SyncInfo(
                        on_wait=keep, on_update=list(si.on_update)
                    )
                new_insts.append(inst)
            bb.instructions[:] = new_insts
    return n


def _install_ntff_hook():
    if "antenv.axon_hooks" in sys.modules:
        return
    try:
        from trn_agent_boot.trn_boot import _ntff_profile_via_ctypes
        hook = _ntff_profile_via_ctypes("/opt/axon/libaxon_pjrt.so")
    except Exception:
        hook = None
    mod = types.ModuleType("antenv.axon_hooks")
    mod.get_axon_ntff_profile_hook = lambda: hook
    mod.set_axon_ntff_profile_hook = lambda h: None
    sys.modules["antenv.axon_hooks"] = mod
    try:
        import antenv
        antenv.axon_hooks = mod
    except Exception:
        pass


def r32(ap):
    return ap.bitcast(F32R)


# --------------------------------------------------------------------------
# device program
# --------------------------------------------------------------------------

def build_program(n_layers=L):
    nc = bass.Bass("TRN2", target_bir_lowering=False, debug=False,
                   enable_asserts=True, num_devices=P)
    io = {}
    io["embT"] = nc.dram_tensor("embT", [D, T_EXT], F32, kind="ExternalInput").ap()
    for nm, sh in [("wq", [L, D, D]), ("wk", [L, D, D]), ("wv", [L, D, D]),
                   ("wo", [L, D, D]), ("wf1", [L, D, FF]), ("wf2", [L, FF, D])]:
        io[nm] = nc.dram_tensor(nm, sh, BF16, kind="ExternalInput").ap()
    io["bias_cols"] = nc.dram_tensor("bias_cols", [128, NB], F32, kind="ExternalInput").ap()
    io["gb_rows"] = nc.dram_tensor("gb_rows", [2, NLN * D], BF16, kind="ExternalInput").ap()
    io["maskT"] = nc.dram_tensor("maskT", [5 * 128, 512], BF16, kind="ExternalInput").ap()
    io["maskf"] = nc.dram_tensor("maskf", [1, T_OWN], BF16, kind="ExternalInput").ap()
    io["onesrow"] = nc.dram_tensor("onesrow", [1, 512], BF16, kind="ExternalInput").ap()
    io["pool_out"] = nc.dram_tensor("pool_out", [128, NJ], F32, kind="ExternalOutput").ap()
    io["xfin"] = nc.dram_tensor("xfin", [128, NJ, T_OWN], F32, kind="ExternalOutput").ap()

    with tile.TileContext(nc) as tc:
        _build_tile_kernel(tc, io, n_layers)
    _split_excess_waits(nc)
    return nc


def _build_tile_kernel(tc, io, n_layers):
    nc = tc.nc
    from contextlib import ExitStack

    ctx = ExitStack()
    with ctx:
        consts = ctx.enter_context(tc.tile_pool(name="consts", bufs=1))
        xn_pool = ctx.enter_context(tc.tile_pool(name="xn", bufs=2))
        r_pool = ctx.enter_context(tc.tile_pool(name="rp", bufs=3))
        xb_pool = ctx.enter_context(tc.tile_pool(name="xb", bufs=1))
        kqa_pool = ctx.enter_context(tc.tile_pool(name="kqa", bufs=1))
        v_pool = ctx.enter_context(tc.tile_pool(name="vp", bufs=1))
        h_pool = ctx.enter_context(tc.tile_pool(name="hp", bufs=2))
        w_pool = ctx.enter_context(tc.tile_pool(name="wp", bufs=3))
        gb_pool = ctx.enter_context(tc.tile_pool(name="gbp", bufs=1))
        em_pool = ctx.enter_context(tc.tile_pool(name="emp", bufs=10))
        tmp_pool = ctx.enter_context(tc.tile_pool(name="tmpp", bufs=2))
        sq_pool = ctx.enter_context(tc.tile_pool(name="sqp", bufs=2))
        vec_pool = ctx.enter_context(tc.tile_pool(name="vecp", bufs=3))
        ao_pool = ctx.enter_context(tc.tile_pool(name="aop", bufs=1))
        acc_pool = ctx.enter_context(tc.tile_pool(name="accp", bufs=1))
        dram_pool = ctx.enter_context(tc.tile_pool(name="dram", bufs=2, space="DRAM"))
        big_ps = ctx.enter_context(tc.tile_pool(name="bigps", bufs=2, space="PSUM"))
        score_ps = ctx.enter_context(tc.tile_pool(name="scoreps", bufs=2, space="PSUM"))
        aps_ps = ctx.enter_context(tc.tile_pool(name="apsps", bufs=2, space="PSUM"))
        bc_ps = ctx.enter_context(tc.tile_pool(name="bcps", bufs=2, space="PSUM"))

        # ---- constants ----
        invd_col = consts.tile([128, 1], BF16)
        nc.vector.memset(invd_col, 1.0 / D)
        ones512 = consts.tile([1, 512], BF16)
        nc.vector.memset(ones512, 1.0)
        ones_row = consts.tile([1, 128], BF16)
        nc.vector.memset(ones_row, 1.0)
        ones64 = consts.tile([1, 64], BF16)
        nc.vector.memset(ones64, 1.0)
        bias_sb = consts.tile([128, NB], F32)
        nc.sync.dma_start(out=bias_sb, in_=io["bias_cols"])
        maskp_sb = consts.tile([128, 5, 512], BF16)
        nc.sync.dma_start(out=maskp_sb,
                          in_=io["maskT"].rearrange("(m p) t -> p m t", p=128))
        maskf_sb = consts.tile([1, T_OWN], BF16)
        nc.sync.dma_start(out=maskf_sb, in_=io["maskf"])
        gb_emb_sb = gb_pool.tile([2, 2 * D], BF16, tag="gb")
        nc.sync.dma_start(out=gb_emb_sb[:, 0:D],
                          in_=io["gb_rows"][:, gb_emb():gb_emb() + D])
        eps_col = consts.tile([1, 1], F32)
        nc.vector.memset(eps_col, 1e-5)

        def bcol(idx):
            return bias_sb[:, idx:idx + 1]

        pid = nc.partition_id()
        lidx6 = ((pid + P - 1) % P) * NJ
        ridx6 = ((pid + 1) % P) * NJ

        # ---------------- layer norm helper ----------------
        def ln_stats_rows(src_j):
            """src_j(j) -> AP f32 [128, 512]. Returns (rstd_row [1,512],
            w2 [2,512]): w2 = [-mean*rstd ; ones]."""
            # stat[0] = mean, stat[64] = E[x^2] (1/D folded into invd_col)
            stat = bc_ps.tile([65, 512], F32, tag="bc")
            for j in range(NJ):
                s = src_j(j)
                s_bf = sq_pool.tile([128, 512], BF16, tag="sbf")
                if j % 2 == 0:
                    nc.vector.tensor_copy(s_bf, s)
                else:
                    nc.scalar.activation(s_bf, s, AF.Copy)
                sq = sq_pool.tile([128, 512], BF16, tag="sq")
                nc.scalar.activation(sq, s, AF.Square)
                nc.tensor.matmul(stat[0:1, :], invd_col, s_bf,
                                 start=(j == 0), stop=(j == NJ - 1),
                                 skip_group_check=True)
                nc.tensor.matmul(stat[64:65, :], invd_col, sq,
                                 start=(j == 0), stop=(j == NJ - 1),
                                 skip_group_check=True)
            m2 = vec_pool.tile([1, 512], F32, tag="vec", bufs=3)
            nc.scalar.activation(m2, stat[0:1, :], AF.Square)
            var = vec_pool.tile([1, 512], F32, tag="vec")
            nc.vector.scalar_tensor_tensor(var, stat[64:65, :], 1.0, m2,
                                           AluOpType.mult, AluOpType.subtract)
            sd = vec_pool.tile([1, 512], F32, tag="vec")
            nc.scalar.activation(sd, var, AF.Sqrt, bias=eps_col)
            rstd_f = vec_pool.tile([1, 512], F32, tag="vec")
            nc.vector.reciprocal(rstd_f, sd)
            rstd = vec_pool.tile([1, 512], BF16, tag="vecb", bufs=3, name="rstd")
            nc.vector.tensor_copy(rstd, rstd_f)
            w2 = vec_pool.tile([2, 512], BF16, tag="vec2", bufs=2)
            # w2 = [-mean*rstd ; ones] (ones row DMA'd: engines can't write
            # a lone partition-1 row)
            nc.vector.scalar_tensor_tensor(w2[0:1, :], stat[0:1, :], -1.0, rstd_f,
                                           AluOpType.mult, AluOpType.mult)
            nc.sync.dma_start(out=w2[1:2, :], in_=io["onesrow"])
            return rstd, w2

        def ln_bcast(gb_sb, goff, j, rstd, w2):
            """A = g_j (x) rstd, B = g_j (x) w + b_j (x) 1  (PSUM [128,512])."""
            a_ps = bc_ps.tile([128, 512], F32, tag="bc")
            nc.tensor.matmul(a_ps, gb_sb[0:1, goff + j * 128:goff + (j + 1) * 128],
                             rstd, start=True, stop=True)
            b_ps = bc_ps.tile([128, 512], F32, tag="bc")
            nc.tensor.matmul(b_ps, gb_sb[:, goff + j * 128:goff + (j + 1) * 128],
                             w2, start=True, stop=True)
            return a_ps, b_ps

        # warmup AllGather: absorbs CC setup + inter-core launch skew off
        # the critical path (overlaps the embedding DMA + LN below)
        wu_i = dram_pool.tile([1, 512], BF16, tag="wui")
        wu_o = dram_pool.tile([P, 512], BF16, tag="wuo", addr_space="Shared")
        nc.sync.dma_start(out=wu_i, in_=io["onesrow"])
        nc.gpsimd.collective_compute(
            "AllGather", AluOpType.bypass,
            replica_groups=[list(range(P))],
            ins=[wu_i.opt()], outs=[wu_o.opt()])

        # ---------------- embedding layer norm (over ext tokens) ----------
        xn = xn_pool.tile([128, NJ, T_EXT], BF16, tag="xn")
        r0 = r_pool.tile([128, NJ, T_OWN], F32, tag="r")

        emb_t = []
        for blk in range(2):
            row = []
            for j in range(NJ):
                t = tmp_pool.tile([128, 512], F32, tag="emb", bufs=6, name=f"emb_{blk}_{j}")
                nc.sync.dma_start(
                    out=t,
                    in_=io["embT"][j * 128:(j + 1) * 128, blk * 512:(blk + 1) * 512])
                row.append(t)
            emb_t.append(row)

        for blk in range(2):
            rstd, w2 = ln_stats_rows(lambda j, blk=blk: emb_t[blk][j])
            for j in range(NJ):
                a_ps, b_ps = ln_bcast(gb_emb_sb, 0, j, rstd, w2)
                t = tmp_pool.tile([128, 512], F32, tag="tmp2")
                nc.vector.tensor_tensor(t, emb_t[blk][j], a_ps, AluOpType.mult)
                nc.vector.tensor_tensor(
                    xn[:, j, blk * 512:(blk + 1) * 512], t, b_ps, AluOpType.add)
                if blk == 0:
                    nc.vector.tensor_tensor(
                        r0[:, j, 0:256], t[:, 256:512], b_ps[:, 256:512], AluOpType.add)
                else:
                    nc.vector.tensor_tensor(
                        r0[:, j, 256:512], t[:, 0:256], b_ps[:, 0:256], AluOpType.add)

        # ---------------- transformer layers ----------------
        for l in range(n_layers):
            wq_sb = w_pool.tile([128, NJ, D], BF16, tag="w768")
            nc.sync.dma_start(out=wq_sb, in_=io["wq"][l].rearrange("(k p) o -> p k o", p=128))
            wk_sb = w_pool.tile([128, NJ, D], BF16, tag="w768")
            nc.sync.dma_start(out=wk_sb, in_=io["wk"][l].rearrange("(k p) o -> p k o", p=128))
            wv_sb = w_pool.tile([128, NJ, D], BF16, tag="w768")
            nc.sync.dma_start(out=wv_sb, in_=io["wv"][l].rearrange("(k p) o -> p k o", p=128))
            gb_sb = gb_pool.tile([2, 2 * D], BF16, tag="gb")
            nc.sync.dma_start(out=gb_sb, in_=io["gb_rows"][:, gb_ln1(l):gb_ln1(l) + 2 * D])

            # -- Q projection (feature-major, own tokens) --
            qT = kqa_pool.tile([128, NJ, T_OWN], BF16, tag="qT")
            for mj in range(NJ):
                ps = big_ps.tile([128, 512], F32, tag="big")
                for kj in range(NJ):
                    nc.tensor.matmul(
                        ps, wq_sb[:, kj, mj * 128:(mj + 1) * 128],
                        xn[:, kj, 256:768],
                        start=(kj == 0), stop=(kj == NJ - 1))
                nc.vector.tensor_scalar(
                    qT[:, mj, :], ps, bcol(col_bq(l, mj)), None, AluOpType.add)

            # -- K projection center (ext tokens [256:768]) --
            kT = kqa_pool.tile([128, NJ, T_EXT], BF16, tag="kT")
            for mj in range(NJ):
                ps = big_ps.tile([128, 512], F32, tag="big")
                for kj in range(NJ):
                    nc.tensor.matmul(
                        ps, wk_sb[:, kj, mj * 128:(mj + 1) * 128],
                        xn[:, kj, 256:768],
                        start=(kj == 0), stop=(kj == NJ - 1))
                nc.vector.tensor_scalar(
                    kT[:, mj, 256:768], ps, bcol(col_bk(l, mj)), None, AluOpType.add)

            # -- V projection center (token tiles 2..5, with ones columns) --
            v_sb = v_pool.tile([128, KB, H, HS], BF16, tag="v")

            def v_proj_tt(tt):
                for ob in range(2):
                    psfull = big_ps.tile([128, 512], F32, tag="big")
                    ps = psfull[:, 0:384]
                    for kj in range(NJ):
                        nc.tensor.matmul(
                            ps, xn[:, kj, tt * 128:(tt + 1) * 128],
                            wv_sb[:, kj, ob * 384:(ob + 1) * 384],
                            start=(kj == 0), stop=(kj == NJ - 1))
                    nc.scalar.activation(
                        v_sb[:, tt, ob * 6:(ob + 1) * 6, 0:DH],
                        ps.rearrange("p (h s) -> p h s", s=DH), AF.Copy)
                nc.vector.memset(v_sb[:, tt, :, DH:HS], 1.0)

            for tt in (2, 3, 4, 5):
                v_proj_tt(tt)

            # -- K projection edges (halo-dependent) --
            for mj in range(NJ):
                ps = big_ps.tile([128, 512], F32, tag="big")
                for kj in range(NJ):
                    nc.tensor.matmul(
                        ps[:, 0:256], wk_sb[:, kj, mj * 128:(mj + 1) * 128],
                        xn[:, kj, 0:256],
                        start=(kj == 0), stop=(kj == NJ - 1),
                        skip_group_check=True)
                for kj in range(NJ):
                    nc.tensor.matmul(
                        ps[:, 256:512], wk_sb[:, kj, mj * 128:(mj + 1) * 128],
                        xn[:, kj, 768:1024],
                        start=(kj == 0), stop=(kj == NJ - 1),
                        skip_group_check=True)
                nc.vector.tensor_scalar(
                    kT[:, mj, 0:256], ps[:, 0:256], bcol(col_bk(l, mj)),
                    None, AluOpType.add)
                nc.vector.tensor_scalar(
                    kT[:, mj, 768:1024], ps[:, 256:512], bcol(col_bk(l, mj)),
                    None, AluOpType.add)

            # -- V projection edges --
            for tt in (0, 1, 6, 7):
                v_proj_tt(tt)

            # -- attention: head pairs, band spans packed into 5 PSUM banks --
            # Each ext k-block kb attends a contiguous q-span (|kg-qg|<=256);
            # spans are packed column-wise into 5 full [128,512] banks so exp
            # and mask-mult run as 5 full-width ops per head.
            # bank entries: (kb, bank_lo, bank_hi); q-span = span_q[kb]
            BANKS = (((3, 0, 512),),
                     ((4, 0, 512),),
                     ((2, 0, 384), (0, 384, 512)),
                     ((5, 0, 384), (7, 384, 512)),
                     ((1, 0, 256), (6, 256, 512)))
            QSPAN = {0: (0, 128), 1: (0, 256), 2: (0, 384), 3: (0, 512),
                     4: (0, 512), 5: (128, 512), 6: (256, 512), 7: (384, 512)}
            attnT = kqa_pool.tile([128, NJ, T_OWN], BF16, tag="attnT")

            def emit_scores(jh):
                ems = {}
                for hh in range(2):
                    po = hh * 64
                    for bi, bank in enumerate(BANKS):
                        ps = score_ps.tile([128, 512], F32, tag="score")
                        for kb, blo, bhi in bank:
                            qlo, qhi = QSPAN[kb]
                            nc.tensor.matmul(
                                ps[:, blo:bhi],
                                kT[po:po + 64, jh, kb * 128:(kb + 1) * 128],
                                qT[po:po + 64, jh, qlo:qhi], start=True, stop=True,
                                skip_group_check=True)
                        em = em_pool.tile([128, 512], BF16, tag="em")
                        nc.scalar.activation(em, ps, AF.Exp)
                        nc.vector.scalar_tensor_tensor(
                            em, em, 1.0, maskp_sb[:, bi, :],
                            AluOpType.mult, AluOpType.mult)
                        for kb, blo, bhi in bank:
                            ems[(hh, kb)] = em[:, blo:bhi]
                return ems

            def emit_av(jh, ems):
                aps2, recs = [], []
                for hh in range(2):
                    h = 2 * jh + hh
                    aps = aps_ps.tile([HS, 512], F32, tag="aps")
                    first = True
                    for bank in BANKS:
                        for kb, blo, bhi in bank:
                            qlo, qhi = QSPAN[kb]
                            nc.tensor.matmul(
                                aps[:, qlo:qhi], v_sb[:, kb, h, :], ems[(hh, kb)],
                                start=first, stop=(kb == 6),
                                skip_group_check=True)
                            first = False
                    aps2.append(aps)
                for hh in range(2):
                    rec_f = vec_pool.tile([1, 512], F32, tag="vec")
                    nc.vector.reciprocal(rec_f, aps2[hh][64:65, :])
                    rec = vec_pool.tile([1, 512], BF16, tag="vecb", bufs=3)
                    nc.vector.tensor_copy(rec, rec_f)
                    recs.append(rec)
                return aps2, recs

            def finish_pair(jh, aps2, recs):
                bc2 = bc_ps.tile([128, 512], F32, tag="bc")
                nc.tensor.matmul(bc2[0:64, :], ones64, recs[0],
                                 start=True, stop=True, skip_group_check=True)
                nc.tensor.matmul(bc2[64:128, :], ones64, recs[1],
                                 start=True, stop=True, skip_group_check=True)
                ao2 = ao_pool.tile([128, 512], F32, tag="ao")
                nc.scalar.activation(ao2[0:64, :], aps2[0][0:64, :], AF.Copy)
                nc.scalar.activation(ao2[64:128, :], aps2[1][0:64, :], AF.Copy)
                nc.vector.tensor_tensor(
                    attnT[:, jh, :], ao2, bc2, AluOpType.mult)

            pending = None
            for jh in range(NJ):
                ems = emit_scores(jh)
                if pending is not None:
                    finish_pair(*pending)
                aps2, recs = emit_av(jh, ems)
                pending = (jh, aps2, recs)
            finish_pair(*pending)

            # -- Wo projection + residual -> r1 --
            wo_sb = w_pool.tile([128, NJ, D], BF16, tag="w768")
            nc.sync.dma_start(out=wo_sb, in_=io["wo"][l].rearrange("(k p) o -> p k o", p=128))
            r1 = r_pool.tile([128, NJ, T_OWN], F32, tag="r")
            for mj in range(NJ):
                ps = big_ps.tile([128, 512], F32, tag="big")
                for kj in range(NJ):
                    nc.tensor.matmul(
                        ps, wo_sb[:, kj, mj * 128:(mj + 1) * 128],
                        attnT[:, kj, :],
                        start=(kj == 0), stop=(kj == NJ - 1))
                nc.vector.scalar_tensor_tensor(
                    r1[:, mj, :], ps, bcol(col_bo(l, mj)), r0[:, mj, :],
                    AluOpType.add, AluOpType.add)

            # -- LN1 -> xn1b (bf16) + xn1f (f32) --
            xn1b = xb_pool.tile([128, NJ, T_OWN], BF16, tag="xn1b")
            xn1f = r_pool.tile([128, NJ, T_OWN], F32, tag="r")
            rstd, w2 = ln_stats_rows(lambda j: r1[:, j, :])
            for j in range(NJ):
                a_ps, b_ps = ln_bcast(gb_sb, 0, j, rstd, w2)
                t = tmp_pool.tile([128, 512], F32, tag="tmp2")
                nc.vector.tensor_tensor(t, r1[:, j, :], a_ps, AluOpType.mult)
                nc.vector.tensor_tensor(xn1f[:, j, :], t, b_ps, AluOpType.add)
                nc.scalar.activation(xn1b[:, j, :], xn1f[:, j, :], AF.Copy)

            # -- FFN (2 halves of 2 quarters; FFN2 accumulates a half in PSUM) --
            r2acc = r_pool.tile([128, NJ, T_OWN], F32, tag="r")
            for half in range(2):
                hqs, wf2s = [], []
                for q in (2 * half, 2 * half + 1):
                    wf1_sb = w_pool.tile([128, NJ, D], BF16, tag="w768")
                    nc.sync.dma_start(
                        out=wf1_sb,
                        in_=io["wf1"][l][:, q * D:(q + 1) * D].rearrange("(k p) o -> p k o", p=128))
                    hq = h_pool.tile([128, NJ, T_OWN], BF16, tag="h")
                    for mj in range(NJ):
                        ps = big_ps.tile([128, 512], F32, tag="big")
                        for kj in range(NJ):
                            nc.tensor.matmul(
                                ps, wf1_sb[:, kj, mj * 128:(mj + 1) * 128],
                                xn1b[:, kj, :],
                                start=(kj == 0), stop=(kj == NJ - 1))
                        nc.scalar.activation(
                            hq[:, mj, :], ps, AF.Gelu,
                            bias=bcol(col_bff1(l, q * NJ + mj)))
                    hqs.append(hq)
                    wf2_sb = w_pool.tile([128, NJ, D], BF16, tag="w768")
                    nc.sync.dma_start(
                        out=wf2_sb,
                        in_=io["wf2"][l][q * D:(q + 1) * D, :].rearrange("(k p) o -> p k o", p=128))
                    wf2s.append(wf2_sb)
                for mj in range(NJ):
                    ps = big_ps.tile([128, 512], F32, tag="big")
                    for qi in range(2):
                        for kj in range(NJ):
                            nc.tensor.matmul(
                                ps, wf2s[qi][:, kj, mj * 128:(mj + 1) * 128],
                                hqs[qi][:, kj, :],
                                start=(qi == 0 and kj == 0),
                                stop=(qi == 1 and kj == NJ - 1))
                    dst = r2acc[:, mj, :]
                    if half == 0:
                        nc.vector.tensor_tensor(dst, ps, xn1f[:, mj, :], AluOpType.add)
                    else:
                        nc.vector.scalar_tensor_tensor(
                            dst, ps, bcol(col_bff2(l, mj)), dst,
                            AluOpType.add, AluOpType.add)

            # -- LN2 -> next xn (+ f32 own) --
            last = (l == n_layers - 1)
            xn_next = None if last else xn_pool.tile([128, NJ, T_EXT], BF16, tag="xn")
            xn2f = r_pool.tile([128, NJ, T_OWN], F32, tag="r")
            rstd, w2 = ln_stats_rows(lambda j: r2acc[:, j, :])
            for j in range(NJ):
                a_ps, b_ps = ln_bcast(gb_sb, D, j, rstd, w2)
                t = tmp_pool.tile([128, 512], F32, tag="tmp2")
                nc.vector.tensor_tensor(t, r2acc[:, j, :], a_ps, AluOpType.mult)
                nc.vector.tensor_tensor(xn2f[:, j, :], t, b_ps, AluOpType.add)
                if not last:
                    nc.scalar.activation(
                        xn_next[:, j, 256:768], xn2f[:, j, :], AF.Copy)

            if not last:
                agi = dram_pool.tile([D, T_OWN], BF16, tag="agi")
                ago = dram_pool.tile([P * D, T_OWN], BF16, tag="ago",
                                     addr_space="Shared")
                nc.sync.dma_start(
                    out=agi.rearrange("(j p) t -> p j t", p=128),
                    in_=xn_next[:, :, 256:768])
                nc.gpsimd.collective_compute(
                    "AllGather", AluOpType.bypass,
                    replica_groups=[list(range(P))],
                    ins=[agi.opt()], outs=[ago.opt()])
                agv = ago.rearrange("(r j p) t -> p (r j) t", j=NJ, p=128)
                nc.sync.dma_start(out=xn_next[:, :, 0:256],
                                  in_=agv[:, bass.ds(lidx6, NJ), 256:512])
                nc.sync.dma_start(out=xn_next[:, :, 768:1024],
                                  in_=agv[:, bass.ds(ridx6, NJ), 0:256])
                xn = xn_next
            r0 = xn2f

        # ---------------- pooling partials + debug out ----------------
        nc.sync.dma_start(out=io["xfin"], in_=r0)
        mb = bc_ps.tile([128, 512], F32, tag="bc")
        nc.tensor.matmul(mb, ones_row, maskf_sb, start=True, stop=True)
        accs = acc_pool.tile([128, NJ], F32, tag="accs")
        for j in range(NJ):
            mskd = tmp_pool.tile([128, 512], F32, tag="tmp2")
            nc.vector.tensor_tensor(mskd, r0[:, j, :], mb, AluOpType.mult)
            scr = sq_pool.tile([128, 512], F32, tag="sq")
            nc.scalar.activation(scr, mskd, AF.Copy, accum_out=accs[:, j:j + 1])
        nc.sync.dma_start(out=io["pool_out"], in_=accs)


# --------------------------------------------------------------------------
# host side
# --------------------------------------------------------------------------

BANKS_H = (((3, 0, 512),),
           ((4, 0, 512),),
           ((2, 0, 384), (0, 384, 512)),
           ((5, 0, 384), (7, 384, 512)),
           ((1, 0, 256), (6, 256, 512)))
QSPAN_H = {0: (0, 128), 1: (0, 256), 2: (0, 384), 3: (0, 512),
           4: (0, 512), 5: (128, 512), 6: (256, 512), 7: (384, 512)}


def _build_masks(attention_mask):
    """[P, 5*128, 512] multiplicative bf16 mask, packed per score bank:
    bank bi columns [blo:bhi] hold k-block kb's mask over its q-span."""
    maskf = np.asarray(attention_mask, np.float32).reshape(S)
    masks = np.zeros((P, 5 * 128, 512), np.float32)
    q = np.arange(512)[None, :]
    for c in range(P):
        kg = c * T_OWN - C + np.arange(KB * 128)[:, None]   # global k token
        qg = c * T_OWN + q                                   # global q token
        valid = (kg >= 0) & (kg < S) & (np.abs(kg - qg) <= W)
        mvals = np.where((kg >= 0) & (kg < S), maskf[np.clip(kg, 0, S - 1)], 0.0)
        full = valid * (mvals > 0)                           # [KB*128, 512]
        for bi, bank in enumerate(BANKS_H):
            for kb, blo, bhi in bank:
                qlo, qhi = QSPAN_H[kb]
                masks[c, bi * 128:(bi + 1) * 128, blo:bhi] = \
                    full[kb * 128:(kb + 1) * 128, qlo:qhi]
    return masks


_cache = {}


def kernel(input_ids, attention_mask, word_emb, pos_emb, emb_g, emb_b,
           Wq, Wk, Wv, Wo, bq, bk, bv, bo, ln1_g, ln1_b,
           Wff1, bff1, Wff2, bff2, ln2_g, ln2_b,
           W1, b1, W2, b2, W3, b3):
    to32 = lambda a: np.ascontiguousarray(np.asarray(a, np.float32))
    tob = lambda a: np.ascontiguousarray(np.asarray(a, np.float32).astype(NPBF16))
    ids = np.asarray(input_ids).reshape(S)
    word_emb, pos_emb = to32(word_emb), to32(pos_emb)
    emb = word_emb[ids] + pos_emb                      # [S, D] host gather
    masks = _build_masks(attention_mask)
    maskf = np.asarray(attention_mask, np.float32).reshape(S)

    scale = 1.0 / np.sqrt(np.float32(DH))
    wq_s = to32(Wq) * scale
    bq_s = to32(bq) * scale

    # fold V bias into O bias: attnT excludes bv, so bo_eff = bv @ Wo + bo
    bo_eff = np.einsum("ld,ldo->lo", to32(bv), to32(Wo)) + to32(bo)

    bias_cols = np.zeros((128, NB), np.float32)
    for l in range(L):
        for j in range(NJ):
            sl = slice(j * 128, (j + 1) * 128)
            bias_cols[:, col_bq(l, j)] = bq_s[l][sl]
            bias_cols[:, col_bk(l, j)] = to32(bk)[l][sl]
            bias_cols[:, col_bo(l, j)] = bo_eff[l][sl]
            bias_cols[:, col_bff2(l, j)] = to32(bff2)[l][sl]
        for j in range(NJF):
            bias_cols[:, col_bff1(l, j)] = to32(bff1)[l][j * 128:(j + 1) * 128]

    gb_rows = np.zeros((2, NLN * D), np.float32)  # cast to bf16 below
    gb_rows[0, gb_emb():gb_emb() + D] = to32(emb_g)
    gb_rows[1, gb_emb():gb_emb() + D] = to32(emb_b)
    for l in range(L):
        gb_rows[0, gb_ln1(l):gb_ln1(l) + D] = to32(ln1_g)[l]
        gb_rows[1, gb_ln1(l):gb_ln1(l) + D] = to32(ln1_b)[l]
        gb_rows[0, gb_ln2(l):gb_ln2(l) + D] = to32(ln2_g)[l]
        gb_rows[1, gb_ln2(l):gb_ln2(l) + D] = to32(ln2_b)[l]

    gb_rows_bf = gb_rows.astype(NPBF16)
    wq_b, wk_b, wv_b, wo_b = tob(wq_s), tob(Wk), tob(Wv), tob(Wo)
    wf1_b, wf2_b = tob(Wff1), tob(Wff2)

    n_layers = int(os.environ.get("KERNEL_LAYERS", L))
    if n_layers not in _cache:
        _cache[n_layers] = build_program(n_layers)
    nc = _cache[n_layers]

    in_maps = []
    for c in range(P):
        lo, hi = c * T_OWN - C, c * T_OWN + T_OWN + C
        e = np.zeros((T_EXT, D), np.float32)
        s0, s1 = max(lo, 0), min(hi, S)
        e[s0 - lo:s1 - lo] = emb[s0:s1]
        in_maps.append({
            "embT": np.ascontiguousarray(e.T),
            "wq": wq_b, "wk": wk_b, "wv": wv_b, "wo": wo_b,
            "wf1": wf1_b, "wf2": wf2_b,
            "bias_cols": bias_cols,
            "gb_rows": gb_rows_bf,
            "maskT": np.ascontiguousarray(masks[c].astype(NPBF16)),
            "maskf": np.ascontiguousarray(
                maskf[c * T_OWN:(c + 1) * T_OWN].reshape(1, T_OWN).astype(NPBF16)),
            "onesrow": np.ones((1, 512), NPBF16),
        })

    trace = os.environ.get("KERNEL_TRACE", "0") == "1"
    if trace:
        _install_ntff_hook()
    res = run_bass_kernel_spmd(nc, in_maps, core_ids=list(range(P)), trace=trace)
    kernel.last_exec_time_ns = res.exec_time_ns
    kernel.last_results = res.results

    pooled = np.zeros(D, np.float64)
    for c in range(P):
        po = np.asarray(res.results[c]["pool_out"], np.float64)   # [128, NJ]
        pooled += po.T.reshape(D)                                 # f = j*128+p
    msum = max(maskf.sum(), 1e-9)
    pooled = (pooled / msum).astype(np.float32)

    h1 = np.maximum(pooled @ to32(W1) + to32(b1), 0)
    h2 = np.maximum(h1 @ to32(W2) + to32(b2), 0)
    pred = (h2 @ to32(W3) + to32(b3))[None].astype(np.float32)
    return pred, pred


kernel.last_exec_time_ns = None
kernel.last_results = None


# revision 28
# speedup vs baseline: 1.1002x; 1.1002x over previous
"""Trainium2 Bass kernel for the sliding-window-attention transformer
(nn_Model_22728966930624).

Sharding: sequence-parallel over 8 NeuronCores. Core c owns tokens
[c*512, (c+1)*512); each layer's K/V are computed over an extended region
with a 256-token halo on each side. Halos are refreshed between layers with
an 8-rank AllGather (bf16) plus partition-id-indexed dynamic DMAs.

v2 changes vs baseline:
- All "broadcast"/stats matmuls (LN stats, mean/rstd broadcast, softmax
  denominator broadcast) run as float32r (1 cycle/row at N>=512) instead of
  fp32 (4 cycles/row).
- LayerNorm gamma/beta are folded into the broadcast matmuls:
  A = g (x) rstd, B = g (x) (-mean*rstd) + b (x) 1, apply = s*A + B
  (2 vector ops per feature tile).
- Attention: per head, 8 full-width [64,128,512] score matmuls (one per
  ext k-block, center blocks deduplicated), one [65,512] PSUM accumulator,
  paired-head denominator broadcast via a K=2 selector fp32r matmul.
- V bias folded into the O-projection bias on the host (bo_eff = bv@Wo+bo).
- Layer reordered for AllGather overlap: Q -> K-center -> V-center ->
  (halo) K-edges -> V-edges -> attention with center k-blocks first.
- Fused epilogues via scalar_tensor_tensor where possible.
"""
import os
import sys
import types

import numpy as np
import ml_dtypes

import concourse.bass as bass
import concourse.mybir as mybir
import concourse.tile as tile
from concourse.alu_op_type import AluOpType
from concourse.bass_utils import run_bass_kernel_spmd

F32 = mybir.dt.float32
F32R = mybir.dt.float32r
BF16 = mybir.dt.bfloat16
AF = mybir.ActivationFunctionType
NPBF16 = ml_dtypes.bfloat16

# model dims
S, D, H, DH, L, FF = 4096, 768, 12, 64, 4, 3072
C, W = 256, 256
P = 8                   # cores
T_OWN = S // P          # 512
T_EXT = T_OWN + 2 * C   # 1024
NJ = D // 128           # 6 feature row-tiles
NJF = FF // 128         # 24
HS = DH + 1             # 65: V head slot width (extra ones column)
KB = 8                  # ext k-blocks of 128 tokens

# bias/gamma column registry (shared host/device)
PER_LAYER_COLS = 72
NB = 12 + L * PER_LAYER_COLS


def col_emb_g(j): return j
def col_emb_b(j): return 6 + j
def lbase(l): return 12 + l * PER_LAYER_COLS
def col_bq(l, j): return lbase(l) + j
def col_bk(l, j): return lbase(l) + 6 + j
def col_bo(l, j): return lbase(l) + 12 + j
def col_bff2(l, j): return lbase(l) + 18 + j
def col_bff1(l, j): return lbase(l) + 24 + j       # j in 0..23


# gb_rows column registry: [2, NLN*768]; row0=gamma, row1=beta
def gb_emb(): return 0
def gb_ln1(l): return (1 + 2 * l) * D
def gb_ln2(l): return (2 + 2 * l) * D


NLN = 1 + 2 * L

_MAX_WAITS = 1


def _split_excess_waits(nc, max_waits=_MAX_WAITS):
    """This walrus build rejects >1 semaphore wait per instruction; move
    extras onto same-engine NoOps inserted just before."""
    n = 0
    for f in nc.m.functions:
        for bb in f.blocks:
            new_insts = []
            for inst in bb.instructions:
                si = inst.sync_info
                if si is not None and si.on_wait and len(si.on_wait) > max_waits:
                    excess = list(si.on_wait[:-max_waits])
                    keep = list(si.on_wait[-max_waits:])
                    for k, w in enumerate(excess):
                        nop = mybir.InstNoOp(name=f"{inst.name}-wsplit{k}")
                        nop.engine = inst.engine
                        nop.sync_info = mybir.SyncInfo(on_wait=[w], on_update=[])
                        new_insts.append(nop)
                        n += 1
                    inst.sync_info = mybir.SyncInfo(
                        on_wait=keep, on_update=list(si.on_update)
                    )
                new_insts.append(inst)
            bb.instructions[:] = new_insts
    return n


def _install_ntff_hook():
    if "antenv.axon_hooks" in sys.modules:
        return
    try:
        from trn_agent_boot.trn_boot import _ntff_profile_via_ctypes
        hook = _ntff_profile_via_ctypes("/opt/axon/libaxon_pjrt.so")
    except Exception:
        hook = None
    mod = types.ModuleType("antenv.axon_hooks")
    mod.get_axon_ntff_profile_hook = lambda: hook
    mod.set_axon_ntff_profile_hook = lambda h: None
    sys.modules["antenv.axon_hooks"] = mod
    try:
        import antenv
        antenv.axon_hooks = mod
    except Exception:
        pass


def r32(ap):
    return ap.bitcast(F32R)


# --------------------------------------------------------------------------
# device program
# --------------------------------------------------------------------------

def build_program(n_layers=L):
    nc = bass.Bass("TRN2", target_bir_lowering=False, debug=False,
                   enable_asserts=True, num_devices=P)
    io = {}
    io["embT"] = nc.dram_tensor("embT", [D, T_EXT], F32, kind="ExternalInput").ap()
    for nm, sh in [("wq", [L, D, D]), ("wk", [L, D, D]), ("wv", [L, D, D]),
                   ("wo", [L, D, D]), ("wf1", [L, D, FF]), ("wf2", [L, FF, D])]:
        io[nm] = nc.dram_tensor(nm, sh, BF16, kind="ExternalInput").ap()
    io["bias_cols"] = nc.dram_tensor("bias_cols", [128, NB], F32, kind="ExternalInput").ap()
    io["gb_rows"] = nc.dram_tensor("gb_rows", [2, NLN * D], BF16, kind="ExternalInput").ap()
    io["maskT"] = nc.dram_tensor("maskT", [5 * 128, 512], BF16, kind="ExternalInput").ap()
    io["maskf"] = nc.dram_tensor("maskf", [1, T_OWN], BF16, kind="ExternalInput").ap()
    io["onesrow"] = nc.dram_tensor("onesrow", [1, 512], BF16, kind="ExternalInput").ap()
    io["pool_out"] = nc.dram_tensor("pool_out", [128, NJ], F32, kind="ExternalOutput").ap()
    io["xfin"] = nc.dram_tensor("xfin", [128, NJ, T_OWN], F32, kind="ExternalOutput").ap()

    with tile.TileContext(nc) as tc:
        _build_tile_kernel(tc, io, n_layers)
    _split_excess_waits(nc)
    return nc


def _build_tile_kernel(tc, io, n_layers):
    nc = tc.nc
    from contextlib import ExitStack

    ctx = ExitStack()
    with ctx:
        consts = ctx.enter_context(tc.tile_pool(name="consts", bufs=1))
        xn_pool = ctx.enter_context(tc.tile_pool(name="xn", bufs=2))
        r_pool = ctx.enter_context(tc.tile_pool(name="rp", bufs=3))
        xb_pool = ctx.enter_context(tc.tile_pool(name="xb", bufs=1))
        kqa_pool = ctx.enter_context(tc.tile_pool(name="kqa", bufs=1))
        v_pool = ctx.enter_context(tc.tile_pool(name="vp", bufs=1))
        h_pool = ctx.enter_context(tc.tile_pool(name="hp", bufs=2))
        w_pool = ctx.enter_context(tc.tile_pool(name="wp", bufs=3))
        gb_pool = ctx.enter_context(tc.tile_pool(name="gbp", bufs=1))
        em_pool = ctx.enter_context(tc.tile_pool(name="emp", bufs=10))
        tmp_pool = ctx.enter_context(tc.tile_pool(name="tmpp", bufs=2))
        sq_pool = ctx.enter_context(tc.tile_pool(name="sqp", bufs=2))
        vec_pool = ctx.enter_context(tc.tile_pool(name="vecp", bufs=3))
        ao_pool = ctx.enter_context(tc.tile_pool(name="aop", bufs=1))
        acc_pool = ctx.enter_context(tc.tile_pool(name="accp", bufs=1))
        dram_pool = ctx.enter_context(tc.tile_pool(name="dram", bufs=2, space="DRAM"))
        big_ps = ctx.enter_context(tc.tile_pool(name="bigps", bufs=2, space="PSUM"))
        score_ps = ctx.enter_context(tc.tile_pool(name="scoreps", bufs=2, space="PSUM"))
        aps_ps = ctx.enter_context(tc.tile_pool(name="apsps", bufs=2, space="PSUM"))
        bc_ps = ctx.enter_context(tc.tile_pool(name="bcps", bufs=2, space="PSUM"))

        # ---- constants ----
        invd_col = consts.tile([128, 1], BF16)
        nc.vector.memset(invd_col, 1.0 / D)
        ones512 = consts.tile([1, 512], BF16)
        nc.vector.memset(ones512, 1.0)
        ones_row = consts.tile([1, 128], BF16)
        nc.vector.memset(ones_row, 1.0)
        ones64 = consts.tile([1, 64], BF16)
        nc.vector.memset(ones64, 1.0)
        bias_sb = consts.tile([128, NB], F32)
        nc.sync.dma_start(out=bias_sb, in_=io["bias_cols"])
        maskp_sb = consts.tile([128, 5, 512], BF16)
        nc.sync.dma_start(out=maskp_sb,
                          in_=io["maskT"].rearrange("(m p) t -> p m t", p=128))
        maskf_sb = consts.tile([1, T_OWN], BF16)
        nc.sync.dma_start(out=maskf_sb, in_=io["maskf"])
        gb_emb_sb = gb_pool.tile([2, 2 * D], BF16, tag="gb")
        nc.sync.dma_start(out=gb_emb_sb[:, 0:D],
                          in_=io["gb_rows"][:, gb_emb():gb_emb() + D])
        eps_col = consts.tile([1, 1], F32)
        nc.vector.memset(eps_col, 1e-5)

        def bcol(idx):
            return bias_sb[:, idx:idx + 1]

        pid = nc.partition_id()
        lidx6 = ((pid + P - 1) % P) * NJ
        ridx6 = ((pid + 1) % P) * NJ

        # ---------------- layer norm helper ----------------
        def ln_stats_rows(src_j):
            """src_j(j) -> AP f32 [128, 512]. Returns (rstd_row [1,512],
            w2 [2,512]): w2 = [-mean*rstd ; ones]."""
            # stat[0] = mean, stat[64] = E[x^2] (1/D folded into invd_col)
            stat = bc_ps.tile([65, 512], F32, tag="bc")
            for j in range(NJ):
                s = src_j(j)
                s_bf = sq_pool.tile([128, 512], BF16, tag="sbf")
                if j % 2 == 0:
                    nc.vector.tensor_copy(s_bf, s)
                else:
                    nc.scalar.activation(s_bf, s, AF.Copy)
                sq = sq_pool.tile([128, 512], BF16, tag="sq")
                nc.scalar.activation(sq, s, AF.Square)
                nc.tensor.matmul(stat[0:1, :], invd_col, s_bf,
                                 start=(j == 0), stop=(j == NJ - 1),
                                 skip_group_check=True)
                nc.tensor.matmul(stat[64:65, :], invd_col, sq,
                                 start=(j == 0), stop=(j == NJ - 1),
                                 skip_group_check=True)
            m2 = vec_pool.tile([1, 512], F32, tag="vec", bufs=3)
            nc.scalar.activation(m2, stat[0:1, :], AF.Square)
            var = vec_pool.tile([1, 512], F32, tag="vec")
            nc.vector.scalar_tensor_tensor(var, stat[64:65, :], 1.0, m2,
                                           AluOpType.mult, AluOpType.subtract)
            # rstd = (var+eps)^-0.5 via Ln/Exp on the scalar engine (the
            # DVE RECIPROCAL ucode op costs ~3.3us per row)
            lnv = vec_pool.tile([1, 512], F32, tag="vec")
            nc.scalar.activation(lnv, var, AF.Ln, bias=eps_col)
            rstd_f = vec_pool.tile([1, 512], F32, tag="vec")
            nc.scalar.activation(rstd_f, lnv, AF.Exp, scale=-0.5)
            rstd = vec_pool.tile([1, 512], BF16, tag="vecb", bufs=3, name="rstd")
            nc.vector.tensor_copy(rstd, rstd_f)
            w2 = vec_pool.tile([2, 512], BF16, tag="vec2", bufs=2)
            # w2 = [-mean*rstd ; ones] (ones row DMA'd: engines can't write
            # a lone partition-1 row)
            nc.vector.scalar_tensor_tensor(w2[0:1, :], stat[0:1, :], -1.0, rstd_f,
                                           AluOpType.mult, AluOpType.mult)
            nc.sync.dma_start(out=w2[1:2, :], in_=io["onesrow"])
            return rstd, w2

        def ln_bcast(gb_sb, goff, j, rstd, w2):
            """A = g_j (x) rstd, B = g_j (x) w + b_j (x) 1  (PSUM [128,512])."""
            a_ps = bc_ps.tile([128, 512], F32, tag="bc")
            nc.tensor.matmul(a_ps, gb_sb[0:1, goff + j * 128:goff + (j + 1) * 128],
                             rstd, start=True, stop=True)
            b_ps = bc_ps.tile([128, 512], F32, tag="bc")
            nc.tensor.matmul(b_ps, gb_sb[:, goff + j * 128:goff + (j + 1) * 128],
                             w2, start=True, stop=True)
            return a_ps, b_ps

        # warmup AllGather: absorbs CC setup + inter-core launch skew off
        # the critical path (overlaps the embedding DMA + LN below)
        wu_i = dram_pool.tile([1, 512], BF16, tag="wui")
        wu_o = dram_pool.tile([P, 512], BF16, tag="wuo", addr_space="Shared")
        nc.sync.dma_start(out=wu_i, in_=io["onesrow"])
        nc.gpsimd.collective_compute(
            "AllGather", AluOpType.bypass,
            replica_groups=[list(range(P))],
            ins=[wu_i.opt()], outs=[wu_o.opt()])

        # ---------------- embedding layer norm (over ext tokens) ----------
        xn = xn_pool.tile([128, NJ, T_EXT], BF16, tag="xn")
        r0 = r_pool.tile([128, NJ, T_OWN], F32, tag="r")

        emb_t = []
        for blk in range(2):
            row = []
            for j in range(NJ):
                t = tmp_pool.tile([128, 512], F32, tag="emb", bufs=6, name=f"emb_{blk}_{j}")
                nc.sync.dma_start(
                    out=t,
                    in_=io["embT"][j * 128:(j + 1) * 128, blk * 512:(blk + 1) * 512])
                row.append(t)
            emb_t.append(row)

        for blk in range(2):
            rstd, w2 = ln_stats_rows(lambda j, blk=blk: emb_t[blk][j])
            for j in range(NJ):
                a_ps, b_ps = ln_bcast(gb_emb_sb, 0, j, rstd, w2)
                t = tmp_pool.tile([128, 512], F32, tag="tmp2")
                nc.vector.tensor_tensor(t, emb_t[blk][j], a_ps, AluOpType.mult)
                nc.vector.tensor_tensor(
                    xn[:, j, blk * 512:(blk + 1) * 512], t, b_ps, AluOpType.add)
                if blk == 0:
                    nc.vector.tensor_tensor(
                        r0[:, j, 0:256], t[:, 256:512], b_ps[:, 256:512], AluOpType.add)
                else:
                    nc.vector.tensor_tensor(
                        r0[:, j, 256:512], t[:, 0:256], b_ps[:, 0:256], AluOpType.add)

        # ---------------- transformer layers ----------------
        for l in range(n_layers):
            wq_sb = w_pool.tile([128, NJ, D], BF16, tag="w768")
            nc.sync.dma_start(out=wq_sb, in_=io["wq"][l].rearrange("(k p) o -> p k o", p=128))
            wk_sb = w_pool.tile([128, NJ, D], BF16, tag="w768")
            nc.sync.dma_start(out=wk_sb, in_=io["wk"][l].rearrange("(k p) o -> p k o", p=128))
            wv_sb = w_pool.tile([128, NJ, D], BF16, tag="w768")
            nc.sync.dma_start(out=wv_sb, in_=io["wv"][l].rearrange("(k p) o -> p k o", p=128))
            gb_sb = gb_pool.tile([2, 2 * D], BF16, tag="gb")
            nc.sync.dma_start(out=gb_sb, in_=io["gb_rows"][:, gb_ln1(l):gb_ln1(l) + 2 * D])

            # -- Q projection (feature-major, own tokens) --
            qT = kqa_pool.tile([128, NJ, T_OWN], BF16, tag="qT")
            for mj in range(NJ):
                ps = big_ps.tile([128, 512], F32, tag="big")
                for kj in range(NJ):
                    nc.tensor.matmul(
                        ps, wq_sb[:, kj, mj * 128:(mj + 1) * 128],
                        xn[:, kj, 256:768],
                        start=(kj == 0), stop=(kj == NJ - 1))
                nc.vector.tensor_scalar(
                    qT[:, mj, :], ps, bcol(col_bq(l, mj)), None, AluOpType.add)

            # -- K projection center (ext tokens [256:768]) --
            kT = kqa_pool.tile([128, NJ, T_EXT], BF16, tag="kT")
            for mj in range(NJ):
                ps = big_ps.tile([128, 512], F32, tag="big")
                for kj in range(NJ):
                    nc.tensor.matmul(
                        ps, wk_sb[:, kj, mj * 128:(mj + 1) * 128],
                        xn[:, kj, 256:768],
                        start=(kj == 0), stop=(kj == NJ - 1))
                nc.vector.tensor_scalar(
                    kT[:, mj, 256:768], ps, bcol(col_bk(l, mj)), None, AluOpType.add)

            # -- V projection center (token tiles 2..5, with ones columns) --
            v_sb = v_pool.tile([128, KB, H, HS], BF16, tag="v")

            def v_proj_tt(tt):
                for ob in range(2):
                    psfull = big_ps.tile([128, 512], F32, tag="big")
                    ps = psfull[:, 0:384]
                    for kj in range(NJ):
                        nc.tensor.matmul(
                            ps, xn[:, kj, tt * 128:(tt + 1) * 128],
                            wv_sb[:, kj, ob * 384:(ob + 1) * 384],
                            start=(kj == 0), stop=(kj == NJ - 1))
                    nc.scalar.activation(
                        v_sb[:, tt, ob * 6:(ob + 1) * 6, 0:DH],
                        ps.rearrange("p (h s) -> p h s", s=DH), AF.Copy)
                nc.vector.memset(v_sb[:, tt, :, DH:HS], 1.0)

            for tt in (2, 3, 4, 5):
                v_proj_tt(tt)

            # -- K projection edges (halo-dependent) --
            for mj in range(NJ):
                ps = big_ps.tile([128, 512], F32, tag="big")
                for kj in range(NJ):
                    nc.tensor.matmul(
                        ps[:, 0:256], wk_sb[:, kj, mj * 128:(mj + 1) * 128],
                        xn[:, kj, 0:256],
                        start=(kj == 0), stop=(kj == NJ - 1),
                        skip_group_check=True)
                for kj in range(NJ):
                    nc.tensor.matmul(
                        ps[:, 256:512], wk_sb[:, kj, mj * 128:(mj + 1) * 128],
                        xn[:, kj, 768:1024],
                        start=(kj == 0), stop=(kj == NJ - 1),
                        skip_group_check=True)
                nc.vector.tensor_scalar(
                    kT[:, mj, 0:256], ps[:, 0:256], bcol(col_bk(l, mj)),
                    None, AluOpType.add)
                nc.vector.tensor_scalar(
                    kT[:, mj, 768:1024], ps[:, 256:512], bcol(col_bk(l, mj)),
                    None, AluOpType.add)

            # -- V projection edges --
            for tt in (0, 1, 6, 7):
                v_proj_tt(tt)

            # -- attention: head pairs, band spans packed into 5 PSUM banks --
            # Each ext k-block kb attends a contiguous q-span (|kg-qg|<=256);
            # spans are packed column-wise into 5 full [128,512] banks so exp
            # and mask-mult run as 5 full-width ops per head.
            # bank entries: (kb, bank_lo, bank_hi); q-span = span_q[kb]
            BANKS = (((3, 0, 512),),
                     ((4, 0, 512),),
                     ((2, 0, 384), (0, 384, 512)),
                     ((5, 0, 384), (7, 384, 512)),
                     ((1, 0, 256), (6, 256, 512)))
            QSPAN = {0: (0, 128), 1: (0, 256), 2: (0, 384), 3: (0, 512),
                     4: (0, 512), 5: (128, 512), 6: (256, 512), 7: (384, 512)}
            attnT = kqa_pool.tile([128, NJ, T_OWN], BF16, tag="attnT")

            def emit_scores(jh):
                ems = {}
                for hh in range(2):
                    po = hh * 64
                    for bi, bank in enumerate(BANKS):
                        ps = score_ps.tile([128, 512], F32, tag="score")
                        for kb, blo, bhi in bank:
                            qlo, qhi = QSPAN[kb]
                            nc.tensor.matmul(
                                ps[:, blo:bhi],
                                kT[po:po + 64, jh, kb * 128:(kb + 1) * 128],
                                qT[po:po + 64, jh, qlo:qhi], start=True, stop=True,
                                skip_group_check=True)
                        e = em_pool.tile([128, 512], BF16, tag="e", bufs=3)
                        nc.scalar.activation(e, ps, AF.Exp)
                        em = em_pool.tile([128, 512], BF16, tag="em")
                        nc.vector.scalar_tensor_tensor(
                            em, e, 1.0, maskp_sb[:, bi, :],
                            AluOpType.mult, AluOpType.mult)
                        for kb, blo, bhi in bank:
                            ems[(hh, kb)] = em[:, blo:bhi]
                return ems

            def emit_av(jh, ems):
                aps2, recs = [], []
                for hh in range(2):
                    h = 2 * jh + hh
                    aps = aps_ps.tile([HS, 512], F32, tag="aps")
                    first = True
                    for bank in BANKS:
                        for kb, blo, bhi in bank:
                            qlo, qhi = QSPAN[kb]
                            nc.tensor.matmul(
                                aps[:, qlo:qhi], v_sb[:, kb, h, :], ems[(hh, kb)],
                                start=first, stop=(kb == 6),
                                skip_group_check=True)
                            first = False
                    aps2.append(aps)
                for hh in range(2):
                    lnd = vec_pool.tile([1, 512], F32, tag="vec")
                    nc.scalar.activation(lnd, aps2[hh][64:65, :], AF.Ln)
                    rec = vec_pool.tile([1, 512], BF16, tag="vecb", bufs=3)
                    nc.scalar.activation(rec, lnd, AF.Exp, scale=-1.0)
                    recs.append(rec)
                return aps2, recs

            def finish_pair(jh, aps2, recs):
                bc2 = bc_ps.tile([128, 512], F32, tag="bc")
                nc.tensor.matmul(bc2[0:64, :], ones64, recs[0],
                                 start=True, stop=True, skip_group_check=True)
                nc.tensor.matmul(bc2[64:128, :], ones64, recs[1],
                                 start=True, stop=True, skip_group_check=True)
                ao2 = ao_pool.tile([128, 512], F32, tag="ao")
                nc.scalar.activation(ao2[0:64, :], aps2[0][0:64, :], AF.Copy)
                nc.scalar.activation(ao2[64:128, :], aps2[1][0:64, :], AF.Copy)
                nc.vector.tensor_tensor(
                    attnT[:, jh, :], ao2, bc2, AluOpType.mult)

            pending = None
            for jh in range(NJ):
                ems = emit_scores(jh)
                if pending is not None:
                    finish_pair(*pending)
                aps2, recs = emit_av(jh, ems)
                pending = (jh, aps2, recs)
            finish_pair(*pending)

            # -- Wo projection + residual -> r1 --
            wo_sb = w_pool.tile([128, NJ, D], BF16, tag="w768")
            nc.sync.dma_start(out=wo_sb, in_=io["wo"][l].rearrange("(k p) o -> p k o", p=128))
            r1 = r_pool.tile([128, NJ, T_OWN], F32, tag="r")
            for mj in range(NJ):
                ps = big_ps.tile([128, 512], F32, tag="big")
                for kj in range(NJ):
                    nc.tensor.matmul(
                        ps, wo_sb[:, kj, mj * 128:(mj + 1) * 128],
                        attnT[:, kj, :],
                        start=(kj == 0), stop=(kj == NJ - 1))
                nc.vector.scalar_tensor_tensor(
                    r1[:, mj, :], ps, bcol(col_bo(l, mj)), r0[:, mj, :],
                    AluOpType.add, AluOpType.add)

            # -- LN1 -> xn1b (bf16) + xn1f (f32) --
            xn1b = xb_pool.tile([128, NJ, T_OWN], BF16, tag="xn1b")
            xn1f = r_pool.tile([128, NJ, T_OWN], F32, tag="r")
            rstd, w2 = ln_stats_rows(lambda j: r1[:, j, :])
            for j in range(NJ):
                a_ps, b_ps = ln_bcast(gb_sb, 0, j, rstd, w2)
                t = tmp_pool.tile([128, 512], F32, tag="tmp2")
                nc.vector.tensor_tensor(t, r1[:, j, :], a_ps, AluOpType.mult)
                nc.vector.tensor_tensor(xn1f[:, j, :], t, b_ps, AluOpType.add)
                nc.scalar.activation(xn1b[:, j, :], xn1f[:, j, :], AF.Copy)

            # -- FFN (2 halves of 2 quarters; FFN2 accumulates a half in PSUM) --
            r2acc = r_pool.tile([128, NJ, T_OWN], F32, tag="r")
            for half in range(2):
                hqs, wf2s = [], []
                for q in (2 * half, 2 * half + 1):
                    wf1_sb = w_pool.tile([128, NJ, D], BF16, tag="w768")
                    nc.sync.dma_start(
                        out=wf1_sb,
                        in_=io["wf1"][l][:, q * D:(q + 1) * D].rearrange("(k p) o -> p k o", p=128))
                    hq = h_pool.tile([128, NJ, T_OWN], BF16, tag="h")
                    for mj in range(NJ):
                        ps = big_ps.tile([128, 512], F32, tag="big")
                        for kj in range(NJ):
                            nc.tensor.matmul(
                                ps, wf1_sb[:, kj, mj * 128:(mj + 1) * 128],
                                xn1b[:, kj, :],
                                start=(kj == 0), stop=(kj == NJ - 1))
                        nc.scalar.activation(
                            hq[:, mj, :], ps, AF.Gelu,
                            bias=bcol(col_bff1(l, q * NJ + mj)))
                    hqs.append(hq)
                    wf2_sb = w_pool.tile([128, NJ, D], BF16, tag="w768")
                    nc.sync.dma_start(
                        out=wf2_sb,
                        in_=io["wf2"][l][q * D:(q + 1) * D, :].rearrange("(k p) o -> p k o", p=128))
                    wf2s.append(wf2_sb)
                for mj in range(NJ):
                    ps = big_ps.tile([128, 512], F32, tag="big")
                    for qi in range(2):
                        for kj in range(NJ):
                            nc.tensor.matmul(
                                ps, wf2s[qi][:, kj, mj * 128:(mj + 1) * 128],
                                hqs[qi][:, kj, :],
                                start=(qi == 0 and kj == 0),
                                stop=(qi == 1 and kj == NJ - 1))
                    dst = r2acc[:, mj, :]
                    if half == 0:
                        nc.vector.tensor_tensor(dst, ps, xn1f[:, mj, :], AluOpType.add)
                    else:
                        nc.vector.scalar_tensor_tensor(
                            dst, ps, bcol(col_bff2(l, mj)), dst,
                            AluOpType.add, AluOpType.add)

            # -- LN2 -> next xn (+ f32 own) --
            last = (l == n_layers - 1)
            xn_next = None if last else xn_pool.tile([128, NJ, T_EXT], BF16, tag="xn")
            xn2f = r_pool.tile([128, NJ, T_OWN], F32, tag="r")
            rstd, w2 = ln_stats_rows(lambda j: r2acc[:, j, :])
            for j in range(NJ):
                a_ps, b_ps = ln_bcast(gb_sb, D, j, rstd, w2)
                t = tmp_pool.tile([128, 512], F32, tag="tmp2")
                nc.vector.tensor_tensor(t, r2acc[:, j, :], a_ps, AluOpType.mult)
                nc.vector.tensor_tensor(xn2f[:, j, :], t, b_ps, AluOpType.add)
                if not last:
                    nc.scalar.activation(
                        xn_next[:, j, 256:768], xn2f[:, j, :], AF.Copy)

            if not last:
                agi = dram_pool.tile([D, T_OWN], BF16, tag="agi")
                ago = dram_pool.tile([P * D, T_OWN], BF16, tag="ago",
                                     addr_space="Shared")
                nc.sync.dma_start(
                    out=agi.rearrange("(j p) t -> p j t", p=128),
                    in_=xn_next[:, :, 256:768])
                nc.gpsimd.collective_compute(
                    "AllGather", AluOpType.bypass,
                    replica_groups=[list(range(P))],
                    ins=[agi.opt()], outs=[ago.opt()])
                agv = ago.rearrange("(r j p) t -> p (r j) t", j=NJ, p=128)
                nc.sync.dma_start(out=xn_next[:, :, 0:256],
                                  in_=agv[:, bass.ds(lidx6, NJ), 256:512])
                nc.sync.dma_start(out=xn_next[:, :, 768:1024],
                                  in_=agv[:, bass.ds(ridx6, NJ), 0:256])
                xn = xn_next
            r0 = xn2f

        # ---------------- pooling partials + debug out ----------------
        nc.sync.dma_start(out=io["xfin"], in_=r0)
        mb = bc_ps.tile([128, 512], F32, tag="bc")
        nc.tensor.matmul(mb, ones_row, maskf_sb, start=True, stop=True)
        accs = acc_pool.tile([128, NJ], F32, tag="accs")
        for j in range(NJ):
            mskd = tmp_pool.tile([128, 512], F32, tag="tmp2")
            nc.vector.tensor_tensor(mskd, r0[:, j, :], mb, AluOpType.mult)
            scr = sq_pool.tile([128, 512], F32, tag="sq")
            nc.scalar.activation(scr, mskd, AF.Copy, accum_out=accs[:, j:j + 1])
        nc.sync.dma_start(out=io["pool_out"], in_=accs)


# --------------------------------------------------------------------------
# host side
# --------------------------------------------------------------------------

BANKS_H = (((3, 0, 512),),
           ((4, 0, 512),),
           ((2, 0, 384), (0, 384, 512)),
           ((5, 0, 384), (7, 384, 512)),
           ((1, 0, 256), (6, 256, 512)))
QSPAN_H = {0: (0, 128), 1: (0, 256), 2: (0, 384), 3: (0, 512),
           4: (0, 512), 5: (128, 512), 6: (256, 512), 7: (384, 512)}


def _build_masks(attention_mask):
    """[P, 5*128, 512] multiplicative bf16 mask, packed per score bank:
    bank bi columns [blo:bhi] hold k-block kb's mask over its q-span."""
    maskf = np.asarray(attention_mask, np.float32).reshape(S)
    masks = np.zeros((P, 5 * 128, 512), np.float32)
    q = np.arange(512)[None, :]
    for c in range(P):
        kg = c * T_OWN - C + np.arange(KB * 128)[:, None]   # global k token
        qg = c * T_OWN + q                                   # global q token
        valid = (kg >= 0) & (kg < S) & (np.abs(kg - qg) <= W)
        mvals = np.where((kg >= 0) & (kg < S), maskf[np.clip(kg, 0, S - 1)], 0.0)
        full = valid * (mvals > 0)                           # [KB*128, 512]
        for bi, bank in enumerate(BANKS_H):
            for kb, blo, bhi in bank:
                qlo, qhi = QSPAN_H[kb]
                masks[c, bi * 128:(bi + 1) * 128, blo:bhi] = \
                    full[kb * 128:(kb + 1) * 128, qlo:qhi]
    return masks


_cache = {}


def kernel(input_ids, attention_mask, word_emb, pos_emb, emb_g, emb_b,
           Wq, Wk, Wv, Wo, bq, bk, bv, bo, ln1_g, ln1_b,
           Wff1, bff1, Wff2, bff2, ln2_g, ln2_b,
           W1, b1, W2, b2, W3, b3):
    to32 = lambda a: np.ascontiguousarray(np.asarray(a, np.float32))
    tob = lambda a: np.ascontiguousarray(np.asarray(a, np.float32).astype(NPBF16))
    ids = np.asarray(input_ids).reshape(S)
    word_emb, pos_emb = to32(word_emb), to32(pos_emb)
    emb = word_emb[ids] + pos_emb                      # [S, D] host gather
    masks = _build_masks(attention_mask)
    maskf = np.asarray(attention_mask, np.float32).reshape(S)

    scale = 1.0 / np.sqrt(np.float32(DH))
    wq_s = to32(Wq) * scale
    bq_s = to32(bq) * scale

    # fold V bias into O bias: attnT excludes bv, so bo_eff = bv @ Wo + bo
    bo_eff = np.einsum("ld,ldo->lo", to32(bv), to32(Wo)) + to32(bo)

    bias_cols = np.zeros((128, NB), np.float32)
    for l in range(L):
        for j in range(NJ):
            sl = slice(j * 128, (j + 1) * 128)
            bias_cols[:, col_bq(l, j)] = bq_s[l][sl]
            bias_cols[:, col_bk(l, j)] = to32(bk)[l][sl]
            bias_cols[:, col_bo(l, j)] = bo_eff[l][sl]
            bias_cols[:, col_bff2(l, j)] = to32(bff2)[l][sl]
        for j in range(NJF):
            bias_cols[:, col_bff1(l, j)] = to32(bff1)[l][j * 128:(j + 1) * 128]

    gb_rows = np.zeros((2, NLN * D), np.float32)  # cast to bf16 below
    gb_rows[0, gb_emb():gb_emb() + D] = to32(emb_g)
    gb_rows[1, gb_emb():gb_emb() + D] = to32(emb_b)
    for l in range(L):
        gb_rows[0, gb_ln1(l):gb_ln1(l) + D] = to32(ln1_g)[l]
        gb_rows[1, gb_ln1(l):gb_ln1(l) + D] = to32(ln1_b)[l]
        gb_rows[0, gb_ln2(l):gb_ln2(l) + D] = to32(ln2_g)[l]
        gb_rows[1, gb_ln2(l):gb_ln2(l) + D] = to32(ln2_b)[l]

    gb_rows_bf = gb_rows.astype(NPBF16)
    wq_b, wk_b, wv_b, wo_b = tob(wq_s), tob(Wk), tob(Wv), tob(Wo)
    wf1_b, wf2_b = tob(Wff1), tob(Wff2)

    n_layers = int(os.environ.get("KERNEL_LAYERS", L))
    if n_layers not in _cache:
        _cache[n_layers] = build_program(n_layers)
    nc = _cache[n_layers]

    in_maps = []
    for c in range(P):
        lo, hi = c * T_OWN - C, c * T_OWN + T_OWN + C
        e = np.zeros((T_EXT, D), np.float32)
        s0, s1 = max(lo, 0), min(hi, S)
        e[s0 - lo:s1 - lo] = emb[s0:s1]
        in_maps.append({
            "embT": np.ascontiguousarray(e.T),
            "wq": wq_b, "wk": wk_b, "wv": wv_b, "wo": wo_b,
            "wf1": wf1_b, "wf2": wf2_b,
            "bias_cols": bias_cols,
            "gb_rows": gb_rows_bf,
            "maskT": np.ascontiguousarray(masks[c].astype(NPBF16)),
            "maskf": np.ascontiguousarray(
                maskf[c * T_OWN:(c + 1) * T_OWN].reshape(1, T_OWN).astype(NPBF16)),
            "onesrow": np.ones((1, 512), NPBF16),
        })

    trace = os.environ.get("KERNEL_TRACE", "0") == "1"
    if trace:
        _install_ntff_hook()
    res = run_bass_kernel_spmd(nc, in_maps, core_ids=list(range(P)), trace=trace)
    kernel.last_exec_time_ns = res.exec_time_ns
    kernel.last_results = res.results

    pooled = np.zeros(D, np.float64)
    for c in range(P):
        po = np.asarray(res.results[c]["pool_out"], np.float64)   # [128, NJ]
        pooled += po.T.reshape(D)                                 # f = j*128+p
    msum = max(maskf.sum(), 1e-9)
    pooled = (pooled / msum).astype(np.float32)

    h1 = np.maximum(pooled @ to32(W1) + to32(b1), 0)
    h2 = np.maximum(h1 @ to32(W2) + to32(b2), 0)
    pred = (h2 @ to32(W3) + to32(b3))[None].astype(np.float32)
    return pred, pred


kernel.last_exec_time_ns = None
kernel.last_results = None


# revision 29
# speedup vs baseline: 1.2004x; 1.0911x over previous
"""Trainium2 Bass kernel for the sliding-window-attention transformer
(nn_Model_22728966930624).

Sharding: sequence-parallel over 8 NeuronCores. Core c owns tokens
[c*512, (c+1)*512); each layer's K/V are computed over an extended region
with a 256-token halo on each side. Halos are refreshed between layers with
an 8-rank AllGather (bf16) plus partition-id-indexed dynamic DMAs.

v2 changes vs baseline:
- All "broadcast"/stats matmuls (LN stats, mean/rstd broadcast, softmax
  denominator broadcast) run as float32r (1 cycle/row at N>=512) instead of
  fp32 (4 cycles/row).
- LayerNorm gamma/beta are folded into the broadcast matmuls:
  A = g (x) rstd, B = g (x) (-mean*rstd) + b (x) 1, apply = s*A + B
  (2 vector ops per feature tile).
- Attention: per head, 8 full-width [64,128,512] score matmuls (one per
  ext k-block, center blocks deduplicated), one [65,512] PSUM accumulator,
  paired-head denominator broadcast via a K=2 selector fp32r matmul.
- V bias folded into the O-projection bias on the host (bo_eff = bv@Wo+bo).
- Layer reordered for AllGather overlap: Q -> K-center -> V-center ->
  (halo) K-edges -> V-edges -> attention with center k-blocks first.
- Fused epilogues via scalar_tensor_tensor where possible.
"""
import os
import sys
import types

import numpy as np
import ml_dtypes

import concourse.bass as bass
import concourse.mybir as mybir
import concourse.tile as tile
from concourse.alu_op_type import AluOpType
from concourse.bass_utils import run_bass_kernel_spmd

F32 = mybir.dt.float32
F32R = mybir.dt.float32r
BF16 = mybir.dt.bfloat16
AF = mybir.ActivationFunctionType
NPBF16 = ml_dtypes.bfloat16

# model dims
S, D, H, DH, L, FF = 4096, 768, 12, 64, 4, 3072
C, W = 256, 256
P = 8                   # cores
T_OWN = S // P          # 512
T_EXT = T_OWN + 2 * C   # 1024
NJ = D // 128           # 6 feature row-tiles
NJF = FF // 128         # 24
HS = DH + 1             # 65: V head slot width (extra ones column)
KB = 8                  # ext k-blocks of 128 tokens

# bias/gamma column registry (shared host/device)
PER_LAYER_COLS = 72
NB = 12 + L * PER_LAYER_COLS


def col_emb_g(j): return j
def col_emb_b(j): return 6 + j
def lbase(l): return 12 + l * PER_LAYER_COLS
def col_bq(l, j): return lbase(l) + j
def col_bk(l, j): return lbase(l) + 6 + j
def col_bo(l, j): return lbase(l) + 12 + j
def col_bff2(l, j): return lbase(l) + 18 + j
def col_bff1(l, j): return lbase(l) + 24 + j       # j in 0..23


# gb_rows column registry: [2, NLN*768]; row0=gamma, row1=beta
def gb_emb(): return 0
def gb_ln1(l): return (1 + 2 * l) * D
def gb_ln2(l): return (2 + 2 * l) * D


NLN = 1 + 2 * L

_MAX_WAITS = 1


def _split_excess_waits(nc, max_waits=_MAX_WAITS):
    """This walrus build rejects >1 semaphore wait per instruction; move
    extras onto same-engine NoOps inserted just before."""
    n = 0
    for f in nc.m.functions:
        for bb in f.blocks:
            new_insts = []
            for inst in bb.instructions:
                si = inst.sync_info
                if si is not None and si.on_wait and len(si.on_wait) > max_waits:
                    excess = list(si.on_wait[:-max_waits])
                    keep = list(si.on_wait[-max_waits:])
                    for k, w in enumerate(excess):
                        nop = mybir.InstNoOp(name=f"{inst.name}-wsplit{k}")
                        nop.engine = inst.engine
                        nop.sync_info = mybir.SyncInfo(on_wait=[w], on_update=[])
                        new_insts.append(nop)
                        n += 1
                    inst.sync_info = mybir.SyncInfo(
                        on_wait=keep, on_update=list(si.on_update)
                    )
                new_insts.append(inst)
            bb.instructions[:] = new_insts
    return n


def _install_ntff_hook():
    if "antenv.axon_hooks" in sys.modules:
        return
    try:
        from trn_agent_boot.trn_boot import _ntff_profile_via_ctypes
        hook = _ntff_profile_via_ctypes("/opt/axon/libaxon_pjrt.so")
    except Exception:
        hook = None
    mod = types.ModuleType("antenv.axon_hooks")
    mod.get_axon_ntff_profile_hook = lambda: hook
    mod.set_axon_ntff_profile_hook = lambda h: None
    sys.modules["antenv.axon_hooks"] = mod
    try:
        import antenv
        antenv.axon_hooks = mod
    except Exception:
        pass


def r32(ap):
    return ap.bitcast(F32R)


# --------------------------------------------------------------------------
# device program
# --------------------------------------------------------------------------

def build_program(n_layers=L):
    nc = bass.Bass("TRN2", target_bir_lowering=False, debug=False,
                   enable_asserts=True, num_devices=P)
    io = {}
    io["embT"] = nc.dram_tensor("embT", [D, T_EXT], F32, kind="ExternalInput").ap()
    for nm, sh in [("wq", [L, D, D]), ("wk", [L, D, D]), ("wv", [L, D, D]),
                   ("wo", [L, D, D]), ("wf1", [L, D, FF]), ("wf2", [L, FF, D])]:
        io[nm] = nc.dram_tensor(nm, sh, BF16, kind="ExternalInput").ap()
    io["bias_cols"] = nc.dram_tensor("bias_cols", [128, NB], F32, kind="ExternalInput").ap()
    io["gb_rows"] = nc.dram_tensor("gb_rows", [2, NLN * D], BF16, kind="ExternalInput").ap()
    io["maskT"] = nc.dram_tensor("maskT", [5 * 128, 512], BF16, kind="ExternalInput").ap()
    io["maskf"] = nc.dram_tensor("maskf", [1, T_OWN], BF16, kind="ExternalInput").ap()
    io["onesrow"] = nc.dram_tensor("onesrow", [1, 512], BF16, kind="ExternalInput").ap()
    io["pool_out"] = nc.dram_tensor("pool_out", [128, NJ], F32, kind="ExternalOutput").ap()
    io["xfin"] = nc.dram_tensor("xfin", [128, NJ, T_OWN], F32, kind="ExternalOutput").ap()

    with tile.TileContext(nc) as tc:
        _build_tile_kernel(tc, io, n_layers)
    _split_excess_waits(nc)
    return nc


def _build_tile_kernel(tc, io, n_layers):
    nc = tc.nc
    from contextlib import ExitStack

    ctx = ExitStack()
    with ctx:
        consts = ctx.enter_context(tc.tile_pool(name="consts", bufs=1))
        xn_pool = ctx.enter_context(tc.tile_pool(name="xn", bufs=2))
        r_pool = ctx.enter_context(tc.tile_pool(name="rp", bufs=3))
        xb_pool = ctx.enter_context(tc.tile_pool(name="xb", bufs=1))
        kqa_pool = ctx.enter_context(tc.tile_pool(name="kqa", bufs=1))
        v_pool = ctx.enter_context(tc.tile_pool(name="vp", bufs=1))
        h_pool = ctx.enter_context(tc.tile_pool(name="hp", bufs=2))
        w_pool = ctx.enter_context(tc.tile_pool(name="wp", bufs=3))
        gb_pool = ctx.enter_context(tc.tile_pool(name="gbp", bufs=1))
        em_pool = ctx.enter_context(tc.tile_pool(name="emp", bufs=10))
        tmp_pool = ctx.enter_context(tc.tile_pool(name="tmpp", bufs=2))
        sq_pool = ctx.enter_context(tc.tile_pool(name="sqp", bufs=2))
        vec_pool = ctx.enter_context(tc.tile_pool(name="vecp", bufs=3))
        ao_pool = ctx.enter_context(tc.tile_pool(name="aop", bufs=1))
        acc_pool = ctx.enter_context(tc.tile_pool(name="accp", bufs=1))
        dram_pool = ctx.enter_context(tc.tile_pool(name="dram", bufs=2, space="DRAM"))
        big_ps = ctx.enter_context(tc.tile_pool(name="bigps", bufs=2, space="PSUM"))
        score_ps = ctx.enter_context(tc.tile_pool(name="scoreps", bufs=2, space="PSUM"))
        aps_ps = ctx.enter_context(tc.tile_pool(name="apsps", bufs=2, space="PSUM"))
        bc_ps = ctx.enter_context(tc.tile_pool(name="bcps", bufs=2, space="PSUM"))

        # ---- constants ----
        invd_col = consts.tile([128, 1], BF16)
        nc.vector.memset(invd_col, 1.0 / D)
        ones512 = consts.tile([1, 512], BF16)
        nc.vector.memset(ones512, 1.0)
        ones_row = consts.tile([1, 128], BF16)
        nc.vector.memset(ones_row, 1.0)
        ones64 = consts.tile([1, 64], BF16)
        nc.vector.memset(ones64, 1.0)
        bias_sb = consts.tile([128, NB], F32)
        nc.sync.dma_start(out=bias_sb, in_=io["bias_cols"])
        maskp_sb = consts.tile([128, 5, 512], BF16)
        nc.sync.dma_start(out=maskp_sb,
                          in_=io["maskT"].rearrange("(m p) t -> p m t", p=128))
        maskf_sb = consts.tile([1, T_OWN], BF16)
        nc.sync.dma_start(out=maskf_sb, in_=io["maskf"])
        gb_emb_sb = gb_pool.tile([2, 2 * D], BF16, tag="gb")
        nc.sync.dma_start(out=gb_emb_sb[:, 0:D],
                          in_=io["gb_rows"][:, gb_emb():gb_emb() + D])
        eps_col = consts.tile([1, 1], F32)
        nc.vector.memset(eps_col, 1e-5)

        def bcol(idx):
            return bias_sb[:, idx:idx + 1]

        pid = nc.partition_id()
        lidx6 = ((pid + P - 1) % P) * NJ
        ridx6 = ((pid + 1) % P) * NJ

        # ---------------- layer norm helper ----------------
        def ln_stats_rows(src_j):
            """src_j(j) -> AP f32 [128, 512]. Returns (rstd_row [1,512],
            w2 [2,512]): w2 = [-mean*rstd ; ones]."""
            # stat[0] = mean, stat[64] = E[x^2] (1/D folded into invd_col)
            stat = bc_ps.tile([65, 512], F32, tag="bc")
            for j in range(NJ):
                s = src_j(j)
                s_bf = sq_pool.tile([128, 512], BF16, tag="sbf")
                if j % 2 == 0:
                    nc.vector.tensor_copy(s_bf, s)
                else:
                    nc.scalar.activation(s_bf, s, AF.Copy)
                sq = sq_pool.tile([128, 512], BF16, tag="sq")
                nc.scalar.activation(sq, s, AF.Square)
                nc.tensor.matmul(stat[0:1, :], invd_col, s_bf,
                                 start=(j == 0), stop=(j == NJ - 1),
                                 skip_group_check=True)
                nc.tensor.matmul(stat[64:65, :], invd_col, sq,
                                 start=(j == 0), stop=(j == NJ - 1),
                                 skip_group_check=True)
            m2 = vec_pool.tile([1, 512], F32, tag="vec", bufs=3)
            nc.scalar.activation(m2, stat[0:1, :], AF.Square)
            var = vec_pool.tile([1, 512], F32, tag="vec")
            nc.vector.scalar_tensor_tensor(var, stat[64:65, :], 1.0, m2,
                                           AluOpType.mult, AluOpType.subtract)
            # rstd = (var+eps)^-0.5 via Ln/Exp on the scalar engine (the
            # DVE RECIPROCAL ucode op costs ~3.3us per row)
            lnv = vec_pool.tile([1, 512], F32, tag="vec")
            nc.scalar.activation(lnv, var, AF.Ln, bias=eps_col)
            rstd_f = vec_pool.tile([1, 512], F32, tag="vec")
            nc.scalar.activation(rstd_f, lnv, AF.Exp, scale=-0.5)
            rstd = vec_pool.tile([1, 512], BF16, tag="vecb", bufs=3, name="rstd")
            nc.vector.tensor_copy(rstd, rstd_f)
            w2 = vec_pool.tile([2, 512], BF16, tag="vec2", bufs=2)
            # w2 = [-mean*rstd ; ones] (ones row DMA'd: engines can't write
            # a lone partition-1 row)
            nc.vector.scalar_tensor_tensor(w2[0:1, :], stat[0:1, :], -1.0, rstd_f,
                                           AluOpType.mult, AluOpType.mult)
            nc.sync.dma_start(out=w2[1:2, :], in_=io["onesrow"])
            return rstd, w2

        def ln_bcast(gb_sb, goff, j, rstd, w2):
            """A = g_j (x) rstd, B = g_j (x) w + b_j (x) 1  (PSUM [128,512])."""
            a_ps = bc_ps.tile([128, 512], F32, tag="bc")
            nc.tensor.matmul(a_ps, gb_sb[0:1, goff + j * 128:goff + (j + 1) * 128],
                             rstd, start=True, stop=True)
            b_ps = bc_ps.tile([128, 512], F32, tag="bc")
            nc.tensor.matmul(b_ps, gb_sb[:, goff + j * 128:goff + (j + 1) * 128],
                             w2, start=True, stop=True)
            return a_ps, b_ps

        # warmup AllGather: absorbs CC setup + inter-core launch skew off
        # the critical path (overlaps the embedding DMA + LN below)
        wu_i = dram_pool.tile([1, 512], BF16, tag="wui")
        wu_o = dram_pool.tile([P, 512], BF16, tag="wuo", addr_space="Shared")
        nc.sync.dma_start(out=wu_i, in_=io["onesrow"])
        nc.gpsimd.collective_compute(
            "AllGather", AluOpType.bypass,
            replica_groups=[list(range(P))],
            ins=[wu_i.opt()], outs=[wu_o.opt()])

        # ---------------- embedding layer norm (over ext tokens) ----------
        xn = xn_pool.tile([128, NJ, T_EXT], BF16, tag="xn")
        r0 = r_pool.tile([128, NJ, T_OWN], F32, tag="r")

        emb_t = []
        for blk in range(2):
            row = []
            for j in range(NJ):
                t = tmp_pool.tile([128, 512], F32, tag="emb", bufs=6, name=f"emb_{blk}_{j}")
                nc.sync.dma_start(
                    out=t,
                    in_=io["embT"][j * 128:(j + 1) * 128, blk * 512:(blk + 1) * 512])
                row.append(t)
            emb_t.append(row)

        for blk in range(2):
            rstd, w2 = ln_stats_rows(lambda j, blk=blk: emb_t[blk][j])
            for j in range(NJ):
                a_ps, b_ps = ln_bcast(gb_emb_sb, 0, j, rstd, w2)
                t = tmp_pool.tile([128, 512], F32, tag="tmp2")
                nc.vector.tensor_tensor(t, emb_t[blk][j], a_ps, AluOpType.mult)
                nc.vector.tensor_tensor(
                    xn[:, j, blk * 512:(blk + 1) * 512], t, b_ps, AluOpType.add)
                if blk == 0:
                    nc.vector.tensor_tensor(
                        r0[:, j, 0:256], t[:, 256:512], b_ps[:, 256:512], AluOpType.add)
                else:
                    nc.vector.tensor_tensor(
                        r0[:, j, 256:512], t[:, 0:256], b_ps[:, 0:256], AluOpType.add)

        # ---------------- transformer layers ----------------
        for l in range(n_layers):
            wq_sb = w_pool.tile([128, NJ, D], BF16, tag="w768")
            nc.sync.dma_start(out=wq_sb, in_=io["wq"][l].rearrange("(k p) o -> p k o", p=128))
            wk_sb = w_pool.tile([128, NJ, D], BF16, tag="w768")
            nc.sync.dma_start(out=wk_sb, in_=io["wk"][l].rearrange("(k p) o -> p k o", p=128))
            wv_sb = w_pool.tile([128, NJ, D], BF16, tag="w768")
            nc.sync.dma_start(out=wv_sb, in_=io["wv"][l].rearrange("(k p) o -> p k o", p=128))
            gb_sb = gb_pool.tile([2, 2 * D], BF16, tag="gb")
            nc.sync.dma_start(out=gb_sb, in_=io["gb_rows"][:, gb_ln1(l):gb_ln1(l) + 2 * D])

            # -- Q projection (feature-major, own tokens) --
            qT = kqa_pool.tile([128, NJ, T_OWN], BF16, tag="qT")
            for mj in range(NJ):
                ps = big_ps.tile([128, 512], F32, tag="big")
                for kj in range(NJ):
                    nc.tensor.matmul(
                        ps, wq_sb[:, kj, mj * 128:(mj + 1) * 128],
                        xn[:, kj, 256:768],
                        start=(kj == 0), stop=(kj == NJ - 1))
                nc.vector.tensor_scalar(
                    qT[:, mj, :], ps, bcol(col_bq(l, mj)), None, AluOpType.add)

            # -- K projection center (ext tokens [256:768]) --
            kT = kqa_pool.tile([128, NJ, T_EXT], BF16, tag="kT")
            for mj in range(NJ):
                ps = big_ps.tile([128, 512], F32, tag="big")
                for kj in range(NJ):
                    nc.tensor.matmul(
                        ps, wk_sb[:, kj, mj * 128:(mj + 1) * 128],
                        xn[:, kj, 256:768],
                        start=(kj == 0), stop=(kj == NJ - 1))
                nc.vector.tensor_scalar(
                    kT[:, mj, 256:768], ps, bcol(col_bk(l, mj)), None, AluOpType.add)

            # -- V projection center (token tiles 2..5, with ones columns) --
            v_sb = v_pool.tile([128, KB, H, HS], BF16, tag="v")

            def v_proj_tt(tt):
                for ob in range(2):
                    psfull = big_ps.tile([128, 512], F32, tag="big")
                    ps = psfull[:, 0:384]
                    for kj in range(NJ):
                        nc.tensor.matmul(
                            ps, xn[:, kj, tt * 128:(tt + 1) * 128],
                            wv_sb[:, kj, ob * 384:(ob + 1) * 384],
                            start=(kj == 0), stop=(kj == NJ - 1))
                    nc.scalar.activation(
                        v_sb[:, tt, ob * 6:(ob + 1) * 6, 0:DH],
                        ps.rearrange("p (h s) -> p h s", s=DH), AF.Copy)
                nc.vector.memset(v_sb[:, tt, :, DH:HS], 1.0)

            for tt in (2, 3, 4, 5):
                v_proj_tt(tt)

            # -- K projection edges (halo-dependent) --
            for mj in range(NJ):
                ps = big_ps.tile([128, 512], F32, tag="big")
                for kj in range(NJ):
                    nc.tensor.matmul(
                        ps[:, 0:256], wk_sb[:, kj, mj * 128:(mj + 1) * 128],
                        xn[:, kj, 0:256],
                        start=(kj == 0), stop=(kj == NJ - 1),
                        skip_group_check=True)
                for kj in range(NJ):
                    nc.tensor.matmul(
                        ps[:, 256:512], wk_sb[:, kj, mj * 128:(mj + 1) * 128],
                        xn[:, kj, 768:1024],
                        start=(kj == 0), stop=(kj == NJ - 1),
                        skip_group_check=True)
                nc.vector.tensor_scalar(
                    kT[:, mj, 0:256], ps[:, 0:256], bcol(col_bk(l, mj)),
                    None, AluOpType.add)
                nc.vector.tensor_scalar(
                    kT[:, mj, 768:1024], ps[:, 256:512], bcol(col_bk(l, mj)),
                    None, AluOpType.add)

            # -- V projection edges --
            for tt in (0, 1, 6, 7):
                v_proj_tt(tt)

            # -- attention: head pairs, band spans packed into 5 PSUM banks --
            # Each ext k-block kb attends a contiguous q-span (|kg-qg|<=256);
            # spans are packed column-wise into 5 full [128,512] banks so exp
            # and mask-mult run as 5 full-width ops per head.
            # bank entries: (kb, bank_lo, bank_hi); q-span = span_q[kb]
            BANKS = (((3, 0, 512),),
                     ((4, 0, 512),),
                     ((2, 0, 384), (0, 384, 512)),
                     ((5, 0, 384), (7, 384, 512)),
                     ((1, 0, 256), (6, 256, 512)))
            QSPAN = {0: (0, 128), 1: (0, 256), 2: (0, 384), 3: (0, 512),
                     4: (0, 512), 5: (128, 512), 6: (256, 512), 7: (384, 512)}
            attnT = kqa_pool.tile([128, NJ, T_OWN], BF16, tag="attnT")

            def emit_scores(jh):
                ems = {}
                for hh in range(2):
                    po = hh * 64
                    for bi, bank in enumerate(BANKS):
                        ps = score_ps.tile([128, 512], F32, tag="score")
                        for kb, blo, bhi in bank:
                            qlo, qhi = QSPAN[kb]
                            nc.tensor.matmul(
                                ps[:, blo:bhi],
                                kT[po:po + 64, jh, kb * 128:(kb + 1) * 128],
                                qT[po:po + 64, jh, qlo:qhi], start=True, stop=True,
                                skip_group_check=True)
                        e = em_pool.tile([128, 512], BF16, tag="e", bufs=3)
                        nc.scalar.activation(e, ps, AF.Exp)
                        em = em_pool.tile([128, 512], BF16, tag="em")
                        nc.vector.scalar_tensor_tensor(
                            em, e, 1.0, maskp_sb[:, bi, :],
                            AluOpType.mult, AluOpType.mult)
                        for kb, blo, bhi in bank:
                            ems[(hh, kb)] = em[:, blo:bhi]
                return ems

            def emit_av(jh, ems):
                aps2, recs = [], []
                for hh in range(2):
                    h = 2 * jh + hh
                    aps = aps_ps.tile([HS, 512], F32, tag="aps")
                    first = True
                    for bank in BANKS:
                        for kb, blo, bhi in bank:
                            qlo, qhi = QSPAN[kb]
                            nc.tensor.matmul(
                                aps[:, qlo:qhi], v_sb[:, kb, h, :], ems[(hh, kb)],
                                start=first, stop=(kb == 6),
                                skip_group_check=True)
                            first = False
                    aps2.append(aps)
                for hh in range(2):
                    lnd = vec_pool.tile([1, 512], F32, tag="vec")
                    nc.scalar.activation(lnd, aps2[hh][64:65, :], AF.Ln)
                    rec = vec_pool.tile([1, 512], BF16, tag="vecb", bufs=3)
                    nc.scalar.activation(rec, lnd, AF.Exp, scale=-1.0)
                    recs.append(rec)
                return aps2, recs

            def finish_pair(jh, aps2, recs):
                bc2 = bc_ps.tile([128, 512], F32, tag="bc")
                nc.tensor.matmul(bc2[0:64, :], ones64, recs[0],
                                 start=True, stop=True, skip_group_check=True)
                nc.tensor.matmul(bc2[64:128, :], ones64, recs[1],
                                 start=True, stop=True, skip_group_check=True)
                ao2 = ao_pool.tile([128, 512], F32, tag="ao")
                nc.vector.tensor_copy(ao2[0:64, :], aps2[0][0:64, :])
                nc.vector.tensor_copy(ao2[64:128, :], aps2[1][0:64, :])
                nc.vector.tensor_tensor(
                    attnT[:, jh, :], ao2, bc2, AluOpType.mult)

            pending = None
            for jh in range(NJ):
                ems = emit_scores(jh)
                if pending is not None:
                    finish_pair(*pending)
                aps2, recs = emit_av(jh, ems)
                pending = (jh, aps2, recs)
            finish_pair(*pending)

            # -- Wo projection + residual -> r1 --
            wo_sb = w_pool.tile([128, NJ, D], BF16, tag="w768")
            nc.sync.dma_start(out=wo_sb, in_=io["wo"][l].rearrange("(k p) o -> p k o", p=128))
            r1 = r_pool.tile([128, NJ, T_OWN], F32, tag="r")
            for mj in range(NJ):
                ps = big_ps.tile([128, 512], F32, tag="big")
                for kj in range(NJ):
                    nc.tensor.matmul(
                        ps, wo_sb[:, kj, mj * 128:(mj + 1) * 128],
                        attnT[:, kj, :],
                        start=(kj == 0), stop=(kj == NJ - 1))
                nc.vector.scalar_tensor_tensor(
                    r1[:, mj, :], ps, bcol(col_bo(l, mj)), r0[:, mj, :],
                    AluOpType.add, AluOpType.add)

            # -- LN1 -> xn1b (bf16) + xn1f (f32) --
            xn1b = xb_pool.tile([128, NJ, T_OWN], BF16, tag="xn1b")
            xn1f = r_pool.tile([128, NJ, T_OWN], F32, tag="r")
            rstd, w2 = ln_stats_rows(lambda j: r1[:, j, :])
            for j in range(NJ):
                a_ps, b_ps = ln_bcast(gb_sb, 0, j, rstd, w2)
                t = tmp_pool.tile([128, 512], F32, tag="tmp2")
                nc.vector.tensor_tensor(t, r1[:, j, :], a_ps, AluOpType.mult)
                nc.vector.tensor_tensor(xn1f[:, j, :], t, b_ps, AluOpType.add)
                nc.vector.tensor_tensor(xn1b[:, j, :], t, b_ps, AluOpType.add)

            # -- FFN (2 halves of 2 quarters; FFN2 accumulates a half in PSUM) --
            r2acc = r_pool.tile([128, NJ, T_OWN], F32, tag="r")
            for half in range(2):
                hqs, wf2s = [], []
                for q in (2 * half, 2 * half + 1):
                    wf1_sb = w_pool.tile([128, NJ, D], BF16, tag="w768")
                    nc.sync.dma_start(
                        out=wf1_sb,
                        in_=io["wf1"][l][:, q * D:(q + 1) * D].rearrange("(k p) o -> p k o", p=128))
                    hq = h_pool.tile([128, NJ, T_OWN], BF16, tag="h")
                    for mj in range(NJ):
                        ps = big_ps.tile([128, 512], F32, tag="big")
                        for kj in range(NJ):
                            nc.tensor.matmul(
                                ps, wf1_sb[:, kj, mj * 128:(mj + 1) * 128],
                                xn1b[:, kj, :],
                                start=(kj == 0), stop=(kj == NJ - 1))
                        nc.scalar.activation(
                            hq[:, mj, :], ps, AF.Gelu,
                            bias=bcol(col_bff1(l, q * NJ + mj)))
                    hqs.append(hq)
                    wf2_sb = w_pool.tile([128, NJ, D], BF16, tag="w768")
                    nc.sync.dma_start(
                        out=wf2_sb,
                        in_=io["wf2"][l][q * D:(q + 1) * D, :].rearrange("(k p) o -> p k o", p=128))
                    wf2s.append(wf2_sb)
                for mj in range(NJ):
                    ps = big_ps.tile([128, 512], F32, tag="big")
                    for qi in range(2):
                        for kj in range(NJ):
                            nc.tensor.matmul(
                                ps, wf2s[qi][:, kj, mj * 128:(mj + 1) * 128],
                                hqs[qi][:, kj, :],
                                start=(qi == 0 and kj == 0),
                                stop=(qi == 1 and kj == NJ - 1))
                    dst = r2acc[:, mj, :]
                    if half == 0:
                        nc.vector.tensor_tensor(dst, ps, xn1f[:, mj, :], AluOpType.add)
                    else:
                        nc.vector.scalar_tensor_tensor(
                            dst, ps, bcol(col_bff2(l, mj)), dst,
                            AluOpType.add, AluOpType.add)

            # -- LN2 -> next xn (+ f32 own) --
            last = (l == n_layers - 1)
            xn_next = None if last else xn_pool.tile([128, NJ, T_EXT], BF16, tag="xn")
            xn2f = r_pool.tile([128, NJ, T_OWN], F32, tag="r")
            rstd, w2 = ln_stats_rows(lambda j: r2acc[:, j, :])
            for j in range(NJ):
                a_ps, b_ps = ln_bcast(gb_sb, D, j, rstd, w2)
                t = tmp_pool.tile([128, 512], F32, tag="tmp2")
                nc.vector.tensor_tensor(t, r2acc[:, j, :], a_ps, AluOpType.mult)
                nc.vector.tensor_tensor(xn2f[:, j, :], t, b_ps, AluOpType.add)
                if not last:
                    nc.scalar.activation(
                        xn_next[:, j, 256:768], xn2f[:, j, :], AF.Copy)

            if not last:
                agi = dram_pool.tile([D, T_OWN], BF16, tag="agi")
                ago = dram_pool.tile([P * D, T_OWN], BF16, tag="ago",
                                     addr_space="Shared")
                nc.sync.dma_start(
                    out=agi.rearrange("(j p) t -> p j t", p=128),
                    in_=xn_next[:, :, 256:768])
                nc.gpsimd.collective_compute(
                    "AllGather", AluOpType.bypass,
                    replica_groups=[list(range(P))],
                    ins=[agi.opt()], outs=[ago.opt()])
                agv = ago.rearrange("(r j p) t -> p (r j) t", j=NJ, p=128)
                nc.sync.dma_start(out=xn_next[:, :, 0:256],
                                  in_=agv[:, bass.ds(lidx6, NJ), 256:512])
                nc.sync.dma_start(out=xn_next[:, :, 768:1024],
                                  in_=agv[:, bass.ds(ridx6, NJ), 0:256])
                xn = xn_next
            r0 = xn2f

        # ---------------- pooling partials + debug out ----------------
        nc.sync.dma_start(out=io["xfin"], in_=r0)
        mb = bc_ps.tile([128, 512], F32, tag="bc")
        nc.tensor.matmul(mb, ones_row, maskf_sb, start=True, stop=True)
        accs = acc_pool.tile([128, NJ], F32, tag="accs")
        for j in range(NJ):
            mskd = tmp_pool.tile([128, 512], F32, tag="tmp2")
            nc.vector.tensor_tensor(mskd, r0[:, j, :], mb, AluOpType.mult)
            scr = sq_pool.tile([128, 512], F32, tag="sq")
            nc.scalar.activation(scr, mskd, AF.Copy, accum_out=accs[:, j:j + 1])
        nc.sync.dma_start(out=io["pool_out"], in_=accs)


# --------------------------------------------------------------------------
# host side
# --------------------------------------------------------------------------

BANKS_H = (((3, 0, 512),),
           ((4, 0, 512),),
           ((2, 0, 384), (0, 384, 512)),
           ((5, 0, 384), (7, 384, 512)),
           ((1, 0, 256), (6, 256, 512)))
QSPAN_H = {0: (0, 128), 1: (0, 256), 2: (0, 384), 3: (0, 512),
           4: (0, 512), 5: (128, 512), 6: (256, 512), 7: (384, 512)}


def _build_masks(attention_mask):
    """[P, 5*128, 512] multiplicative bf16 mask, packed per score bank:
    bank bi columns [blo:bhi] hold k-block kb's mask over its q-span."""
    maskf = np.asarray(attention_mask, np.float32).reshape(S)
    masks = np.zeros((P, 5 * 128, 512), np.float32)
    q = np.arange(512)[None, :]
    for c in range(P):
        kg = c * T_OWN - C + np.arange(KB * 128)[:, None]   # global k token
        qg = c * T_OWN + q                                   # global q token
        valid = (kg >= 0) & (kg < S) & (np.abs(kg - qg) <= W)
        mvals = np.where((kg >= 0) & (kg < S), maskf[np.clip(kg, 0, S - 1)], 0.0)
        full = valid * (mvals > 0)                           # [KB*128, 512]
        for bi, bank in enumerate(BANKS_H):
            for kb, blo, bhi in bank:
                qlo, qhi = QSPAN_H[kb]
                masks[c, bi * 128:(bi + 1) * 128, blo:bhi] = \
                    full[kb * 128:(kb + 1) * 128, qlo:qhi]
    return masks


_cache = {}


def kernel(input_ids, attention_mask, word_emb, pos_emb, emb_g, emb_b,
           Wq, Wk, Wv, Wo, bq, bk, bv, bo, ln1_g, ln1_b,
           Wff1, bff1, Wff2, bff2, ln2_g, ln2_b,
           W1, b1, W2, b2, W3, b3):
    to32 = lambda a: np.ascontiguousarray(np.asarray(a, np.float32))
    tob = lambda a: np.ascontiguousarray(np.asarray(a, np.float32).astype(NPBF16))
    ids = np.asarray(input_ids).reshape(S)
    word_emb, pos_emb = to32(word_emb), to32(pos_emb)
    emb = word_emb[ids] + pos_emb                      # [S, D] host gather
    masks = _build_masks(attention_mask)
    maskf = np.asarray(attention_mask, np.float32).reshape(S)

    scale = 1.0 / np.sqrt(np.float32(DH))
    wq_s = to32(Wq) * scale
    bq_s = to32(bq) * scale

    # fold V bias into O bias: attnT excludes bv, so bo_eff = bv @ Wo + bo
    bo_eff = np.einsum("ld,ldo->lo", to32(bv), to32(Wo)) + to32(bo)

    bias_cols = np.zeros((128, NB), np.float32)
    for l in range(L):
        for j in range(NJ):
            sl = slice(j * 128, (j + 1) * 128)
            bias_cols[:, col_bq(l, j)] = bq_s[l][sl]
            bias_cols[:, col_bk(l, j)] = to32(bk)[l][sl]
            bias_cols[:, col_bo(l, j)] = bo_eff[l][sl]
            bias_cols[:, col_bff2(l, j)] = to32(bff2)[l][sl]
        for j in range(NJF):
            bias_cols[:, col_bff1(l, j)] = to32(bff1)[l][j * 128:(j + 1) * 128]

    gb_rows = np.zeros((2, NLN * D), np.float32)  # cast to bf16 below
    gb_rows[0, gb_emb():gb_emb() + D] = to32(emb_g)
    gb_rows[1, gb_emb():gb_emb() + D] = to32(emb_b)
    for l in range(L):
        gb_rows[0, gb_ln1(l):gb_ln1(l) + D] = to32(ln1_g)[l]
        gb_rows[1, gb_ln1(l):gb_ln1(l) + D] = to32(ln1_b)[l]
        gb_rows[0, gb_ln2(l):gb_ln2(l) + D] = to32(ln2_g)[l]
        gb_rows[1, gb_ln2(l):gb_ln2(l) + D] = to32(ln2_b)[l]

    gb_rows_bf = gb_rows.astype(NPBF16)
    wq_b, wk_b, wv_b, wo_b = tob(wq_s), tob(Wk), tob(Wv), tob(Wo)
    wf1_b, wf2_b = tob(Wff1), tob(Wff2)

    n_layers = int(os.environ.get("KERNEL_LAYERS", L))
    if n_layers not in _cache:
        _cache[n_layers] = build_program(n_layers)
    nc = _cache[n_layers]

    in_maps = []
    for c in range(P):
        lo, hi = c * T_OWN - C, c * T_OWN + T_OWN + C
        e = np.zeros((T_EXT, D), np.float32)
        s0, s1 = max(lo, 0), min(hi, S)
        e[s0 - lo:s1 - lo] = emb[s0:s1]
        in_maps.append({
            "embT": np.ascontiguousarray(e.T),
            "wq": wq_b, "wk": wk_b, "wv": wv_b, "wo": wo_b,
            "wf1": wf1_b, "wf2": wf2_b,
            "bias_cols": bias_cols,
            "gb_rows": gb_rows_bf,
            "maskT": np.ascontiguousarray(masks[c].astype(NPBF16)),
            "maskf": np.ascontiguousarray(
                maskf[c * T_OWN:(c + 1) * T_OWN].reshape(1, T_OWN).astype(NPBF16)),
            "onesrow": np.ones((1, 512), NPBF16),
        })

    trace = os.environ.get("KERNEL_TRACE", "0") == "1"
    if trace:
        _install_ntff_hook()
    res = run_bass_kernel_spmd(nc, in_maps, core_ids=list(range(P)), trace=trace)
    kernel.last_exec_time_ns = res.exec_time_ns
    kernel.last_results = res.results

    pooled = np.zeros(D, np.float64)
    for c in range(P):
        po = np.asarray(res.results[c]["pool_out"], np.float64)   # [128, NJ]
        pooled += po.T.reshape(D)                                 # f = j*128+p
    msum = max(maskf.sum(), 1e-9)
    pooled = (pooled / msum).astype(np.float32)

    h1 = np.maximum(pooled @ to32(W1) + to32(b1), 0)
    h2 = np.maximum(h1 @ to32(W2) + to32(b2), 0)
    pred = (h2 @ to32(W3) + to32(b3))[None].astype(np.float32)
    return pred, pred


kernel.last_exec_time_ns = None
kernel.last_results = None
